# revision 15
# baseline (speedup 1.0000x reference)
"""Trainium2 Bass kernel for nn_MultiHeadQuery_selfattention.

Sharding: 8 cores = 4 batches x 2 lk-halves. Core c handles batch b=c//2 and
key rows L = [128*(c%2), 128*(c%2)+128). Each core computes its output slice
out[b, :, L, :] fully independently (no collectives). A single NEFF serves
all cores: for the upper half, the host swaps the two lk-halves of k and v
(softmax/attention sums are permutation-invariant over keys), so on device
"local lk 0..127" is always the core's half.

Math per core (restructured from the reference):
  khT/vhT/qhT/quT: per-head projections kept transposed [head_dim, token]
  attn_h  = softmax(qh_h @ kh_h.T / 8)          rows = my 128 lk
  out_h   = attn_h @ vh_h                       [128, 64]
  outW    = concat_h(out_h) @ fc_w.T            [128, 512]
  base    = outW + q[b, L, :] + fc_b            [128, 512]
  vW_h    = vh_h[L] @ fc_w.T[64h:64h+64]        [128, 512]
  qattn_h = softmax(qu_h @ kh_h.T / 8)          [64, 256]
  res[nq, l, m] = base[l, m] + sum_h qattn[h, nq, l] * vW[h, l, m]
  out = layernorm(res) * ln_w + ln_b

res is produced on the PE as 128 small matmuls with K=9: stationary =
[qattn rows; ones], moving = [vW rows; base], so PSUM holds res directly.
lk pairs share one PSUM bank [128, 512] = (2 lk x 64 nq) rows; LN stats via
bn_stats/bn_aggr; normalize on ACT. QB/VB hold the K=9 operands in 4
partition groups (local lk l -> partitions 32*(l//32)+{0..8}, slot l%32).
"""

import numpy as np

N_HEAD, D_MODEL, D_K, D_V = 8, 512, 64, 64
B, LEN_K, NQ = 4, 256, 64
LN_EPS = 1e-5
N_CORES = 8

_CACHE = {}


def _build(apply_affine: bool):
    import contextlib

    import concourse.tile as tile
    from concourse import bacc, mybir

    f32 = mybir.dt.float32
    Alu = mybir.AluOpType
    Act = mybir.ActivationFunctionType

    nc = bacc.Bacc("TRN2", target_bir_lowering=False, debug=False,
                   num_devices=N_CORES)

    def din(name, shape):
        return nc.dram_tensor(name, shape, f32, kind="ExternalInput")

    # device-layout inputs ([512, N] host tensors stored as [128, 4, N])
    kT = din("kT", [128, 4, 256])
    vT = din("vT", [128, 4, 256])
    qT = din("qT", [128, 4, 128])     # only my 128 lq columns
    quT = din("quT", [128, 4, 64])
    wq = din("wq", [128, 4, 512])     # w_qs.T
    wk = din("wk", [128, 4, 512])
    wv = din("wv", [128, 4, 512])
    wu = din("wu", [128, 4, 512])
    fwT = din("fwT", [128, 4, 512])   # fc_w.T
    qpb = din("qpb", [128, 512])      # q[b, L, :] + fc_b
    ident = din("ident", [128, 128])
    onesd = din("ones", [32, 64])
    if apply_affine:
        lnw_r = din("lnw_r", [128, 512])
        lnb_r = din("lnb_r", [128, 512])
    # pass u, column-block g holds rows (lkk, nq) of lk = 32g + 2u + lkk
    outd = nc.dram_tensor("out", [16, 128, 2048], f32, kind="ExternalOutput")

    with tile.TileContext(nc) as tc, contextlib.ExitStack() as ctx:
        const = ctx.enter_context(tc.tile_pool(name="const", bufs=1))
        stage = ctx.enter_context(tc.tile_pool(name="stage", bufs=4))
        small = ctx.enter_context(tc.tile_pool(name="small", bufs=8))
        pp_cm = tc.tile_pool(name="pp", bufs=8, space="PSUM")
        pp = pp_cm.__enter__()

        def load(dram, shape):
            t = const.tile(shape, f32, tag=dram.name)
            nc.sync.dma_start(t[:], dram[:])
            return t

        kT_s = load(kT, [128, 4, 256])
        vT_s = load(vT, [128, 4, 256])
        qT_s = load(qT, [128, 4, 128])
        quT_s = load(quT, [128, 4, 64])
        wq_s = load(wq, [128, 4, 512])
        wk_s = load(wk, [128, 4, 512])
        wv_s = load(wv, [128, 4, 512])
        wu_s = load(wu, [128, 4, 512])
        fwT_s = load(fwT, [128, 4, 512])
        qpb_s = load(qpb, [128, 512])
        id_s = load(ident, [128, 128])
        if apply_affine:
            lnw_s = load(lnw_r, [128, 512])
            lnb_s = load(lnb_r, [128, 512])

        # ---- projections: xhT[hd, tok] = sum_dm w.T[dm, hd] * x.T[dm, tok]
        def project(w_s, x_s, ncols, tag):
            dst = const.tile([128, 4, ncols], f32, tag=tag)
            for mo in range(4):
                ps = pp.tile([128, ncols], f32, tag="ps")
                for ki in range(4):
                    nc.tensor.matmul(
                        ps[:], w_s[:, ki, mo * 128:(mo + 1) * 128],
                        x_s[:, ki, :], start=(ki == 0), stop=(ki == 3))
                nc.scalar.copy(dst[:, mo, :], ps[:])
            return dst

        khT = project(wk_s, kT_s, 256, "khT")
        vhT = project(wv_s, vT_s, 256, "vhT")
        qhT = project(wq_s, qT_s, 128, "qhT")
        quhT = project(wu_s, quT_s, 64, "quhT")

        # ---- vh untransposed [lk, hd] as two [128, 512] tiles
        vh = [const.tile([128, 512], f32, name=f"vh{j}", tag=f"vh{j}")
              for j in range(2)]
        for lj in range(2):
            for hc in range(4):
                ps = pp.tile([128, 128], f32, tag="ps")
                nc.tensor.transpose(
                    ps[:], vhT[:, hc, lj * 128:(lj + 1) * 128], id_s[:])
                nc.scalar.copy(vh[lj][:, hc * 128:(hc + 1) * 128], ps[:])

        # ---- per-head attention
        outT = const.tile([128, 4, 128], f32, tag="outT")  # [hd, lq]
        eps_t = const.tile([128, 1], f32, tag="eps_t")
        nc.vector.memset(eps_t[:], LN_EPS)
        VB = const.tile([128, 32, 512], f32, tag="VB")
        QB = const.tile([128, 32, 64], f32, tag="QB")
        for g in range(4):
            nc.sync.dma_start(QB[32 * g + 8: 32 * g + 9, :, :], onesd[:])

        for h in range(8):
            po = (h % 2) * 64   # partition offset of head h inside chunk
            hc = h // 2         # dm/hd chunk index
            # scores [128 lq, 256 lk]
            ps_s = pp.tile([128, 256], f32, tag="ps")
            nc.tensor.matmul(ps_s[:], qhT[po:po + 64, hc, :],
                             khT[po:po + 64, hc, :], start=True, stop=True)
            nmax8 = small.tile([128, 1], f32, tag="nmax8")
            nc.vector.tensor_reduce(nmax8[:], ps_s[:], mybir.AxisListType.X,
                                    Alu.max, negate=True)
            nc.vector.tensor_scalar_mul(nmax8[:], nmax8[:], 0.125)
            aexp = stage.tile([128, 256], f32, tag="aexp")
            rsum = small.tile([128, 1], f32, tag="rsum")
            nc.scalar.activation(aexp[:], ps_s[:], Act.Exp,
                                 bias=nmax8[:], scale=0.125, accum_out=rsum[:])
            rrec = small.tile([128, 1], f32, tag="rrec")
            nc.vector.reciprocal(rrec[:], rsum[:])
            # transpose attnexp first, then out_h = attnexp @ vh_h
            aT = []
            for lj in range(2):
                aT_ps = pp.tile([128, 128], f32, tag="ps")
                nc.tensor.transpose(
                    aT_ps[:], aexp[:, lj * 128:(lj + 1) * 128], id_s[:])
                aT_sb = stage.tile([128, 128], f32, tag=f"aT{lj}")
                nc.scalar.copy(aT_sb[:], aT_ps[:])
                aT.append(aT_sb)
            out_ps = pp.tile([128, 64], f32, tag="ps")
            for lj in range(2):
                nc.tensor.matmul(out_ps[:], aT[lj][:],
                                 vh[lj][:, h * 64:(h + 1) * 64],
                                 start=(lj == 0), stop=(lj == 1))
            outn = stage.tile([128, 64], f32, tag="outn")
            nc.vector.tensor_scalar_mul(outn[:], out_ps[:], rrec[:])
            oT_ps = pp.tile([64, 128], f32, tag="ps")
            nc.tensor.transpose(oT_ps[:], outn[:], id_s[:])
            nc.scalar.copy(outT[po:po + 64, hc, :], oT_ps[:])

            # ---- vW_h -> VB rows h (4 partition groups)
            vw_ps = pp.tile([128, 512], f32, tag="ps")
            nc.tensor.matmul(vw_ps[:], vhT[po:po + 64, hc, 0:128],
                             fwT_s[po:po + 64, hc, :], start=True, stop=True)
            vw_sb = stage.tile([128, 512], f32, tag="vw_sb")
            nc.scalar.copy(vw_sb[:], vw_ps[:])
            for g in range(4):
                nc.sync.dma_start(VB[32 * g + h: 32 * g + h + 1, :, :],
                                  vw_sb[32 * g: 32 * g + 32, :])

            # ---- qattn_h -> QB rows h
            ps_q = pp.tile([64, 256], f32, tag="ps")
            nc.tensor.matmul(ps_q[:], quhT[po:po + 64, hc, :],
                             khT[po:po + 64, hc, :], start=True, stop=True)
            qnmax8 = small.tile([64, 1], f32, tag="qnmax8")
            nc.vector.tensor_reduce(qnmax8[:], ps_q[:], mybir.AxisListType.X,
                                    Alu.max, negate=True)
            nc.vector.tensor_scalar_mul(qnmax8[:], qnmax8[:], 0.125)
            qaexp = stage.tile([64, 256], f32, tag="qaexp")
            qrsum = small.tile([64, 1], f32, tag="qrsum")
            nc.scalar.activation(qaexp[:], ps_q[:], Act.Exp,
                                 bias=qnmax8[:], scale=0.125,
                                 accum_out=qrsum[:])
            qrrec = small.tile([64, 1], f32, tag="qrrec")
            nc.vector.reciprocal(qrrec[:], qrsum[:])
            nc.vector.tensor_scalar_mul(qaexp[:], qaexp[:], qrrec[:])
            qaT_ps = pp.tile([128, 64], f32, tag="ps")
            nc.tensor.transpose(qaT_ps[:], qaexp[:, 0:128], id_s[:64, :64])
            qa_sb = stage.tile([128, 64], f32, tag="qa_sb")
            nc.vector.tensor_copy(qa_sb[:], qaT_ps[:])
            for g in range(4):
                nc.sync.dma_start(QB[32 * g + h: 32 * g + h + 1, :, :],
                                  qa_sb[32 * g: 32 * g + 32, :])

        # ---- outW + base
        ow_ps = pp.tile([128, 512], f32, tag="ps")
        for c in range(4):
            nc.tensor.matmul(ow_ps[:], outT[:, c, :], fwT_s[:, c, :],
                             start=(c == 0), stop=(c == 3))
        base = const.tile([128, 512], f32, tag="base")
        nc.vector.tensor_add(base[:], ow_ps[:], qpb_s[:])
        for g in range(4):
            nc.sync.dma_start(VB[32 * g + 8: 32 * g + 9, :, :],
                              base[32 * g: 32 * g + 32, :])

        # ---- termB + layernorm: 16 passes, 8 lk each (4 row x 2 col tiles)
        pp_cm.__exit__(None, None, None)
        with tc.tile_pool(name="pb", bufs=8, space="PSUM") as pb:
            for u in range(16):
                banks = []
                for g in range(4):
                    bank = pb.tile([128, 512], f32, name=f"bank{u}_{g}",
                                   tag="bank")
                    banks.append(bank)
                    for c in range(2):
                        s = 2 * u + c
                        nc.tensor.matmul(
                            bank[64 * c: 64 * c + 64, :],
                            QB[32 * g: 32 * g + 9, s, :],
                            VB[32 * g: 32 * g + 9, s, :],
                            start=True, stop=True,
                            tile_position=(32 * g, 64 * c))
                st6 = small.tile([128, 6, 4], f32, tag="st6")
                st2 = small.tile([128, 2, 4], f32, tag="st2")
                for g in range(4):
                    nc.vector.bn_stats(st6[:, :, g:g + 1], banks[g][:])
                    nc.vector.bn_aggr(st2[:, :, g:g + 1], st6[:, :, g:g + 1])
                std4 = small.tile([128, 4], f32, tag="std4")
                nc.scalar.activation(std4[:], st2[:, 1, :], Act.Sqrt,
                                     bias=eps_t[:], scale=1.0)
                rstd4 = small.tile([128, 4], f32, tag="rstd4")
                nc.vector.reciprocal(rstd4[:], std4[:])
                nmr4 = small.tile([128, 4], f32, tag="nmr4")
                nc.vector.scalar_tensor_tensor(nmr4[:], st2[:, 0, :], -1.0,
                                               rstd4[:], op0=Alu.mult,
                                               op1=Alu.mult)
                big = stage.tile([128, 2048], f32, tag="big")
                for g in range(4):
                    nc.scalar.activation(big[:, 512 * g: 512 * g + 512],
                                         banks[g][:], Act.Identity,
                                         bias=nmr4[:, g:g + 1],
                                         scale=rstd4[:, g:g + 1])
                    if apply_affine:
                        nc.vector.tensor_mul(big[:, 512 * g: 512 * g + 512],
                                             big[:, 512 * g: 512 * g + 512],
                                             lnw_s[:])
                        nc.vector.tensor_add(big[:, 512 * g: 512 * g + 512],
                                             big[:, 512 * g: 512 * g + 512],
                                             lnb_s[:])
                nc.sync.dma_start(outd[u], big[:])

    nc.compile()
    return nc


def _get_nc(apply_affine: bool):
    if apply_affine not in _CACHE:
        _CACHE[apply_affine] = _build(apply_affine)
    return _CACHE[apply_affine]


def _decode_out(raw):
    # [16, 128, 2048] -> [nq, local lk, m]
    arr = np.asarray(raw).reshape(16, 2, 64, 4, 512)
    return arr.transpose(2, 3, 0, 1, 4).reshape(NQ, 128, 512)


def _to_dev(x):
    # [512, N] -> [128, 4, N]
    return np.ascontiguousarray(
        x.reshape(4, 128, x.shape[1]).transpose(1, 0, 2))


def make_in_maps(q, k, v, query, w_qs, w_ks, w_vs, w_query, fc_w, fc_b,
                 ln_w, ln_b, apply_affine):
    ident = np.eye(128, dtype=np.float32)
    wqT = _to_dev(w_qs.T)
    wkT = _to_dev(w_ks.T)
    wvT = _to_dev(w_vs.T)
    wuT = _to_dev(w_query.T)
    fwT = _to_dev(fc_w.T)
    in_maps = []
    for c in range(N_CORES):
        b, half = c // 2, c % 2
        L0 = 128 * half
        # swap lk halves so that local lk 0..127 is always this core's half
        perm = np.r_[L0:L0 + 128, (128 - L0):(128 - L0) + 128]
        m = {
            "kT": _to_dev(np.ascontiguousarray(k[b][perm].T)),
            "vT": _to_dev(np.ascontiguousarray(v[b][perm].T)),
            "qT": _to_dev(np.ascontiguousarray(q[b, L0:L0 + 128].T)),
            "quT": _to_dev(query[b].T),
            "wq": wqT, "wk": wkT, "wv": wvT, "wu": wuT, "fwT": fwT,
            "qpb": q[b, L0:L0 + 128] + fc_b,
            "ident": ident,
            "ones": np.ones((32, 64), np.float32),
        }
        if apply_affine:
            m["lnw_r"] = np.ascontiguousarray(
                np.broadcast_to(ln_w, (128, 512)))
            m["lnb_r"] = np.ascontiguousarray(
                np.broadcast_to(ln_b, (128, 512)))
        in_maps.append(m)
    return in_maps


def kernel(q, k, v, query, w_qs, w_ks, w_vs, w_query, fc_w, fc_b, ln_w, ln_b):
    from concourse.bass_utils import run_bass_kernel_spmd

    q, k, v, query = (np.asarray(a, np.float32) for a in (q, k, v, query))
    w_qs, w_ks, w_vs, w_query, fc_w = (
        np.asarray(a, np.float32) for a in (w_qs, w_ks, w_vs, w_query, fc_w))
    fc_b, ln_w, ln_b = (np.asarray(a, np.float32)
                        for a in (fc_b, ln_w, ln_b))

    apply_affine = not (np.all(ln_w == 1.0) and np.all(ln_b == 0.0))
    nc = _get_nc(apply_affine)
    in_maps = make_in_maps(q, k, v, query, w_qs, w_ks, w_vs, w_query,
                           fc_w, fc_b, ln_w, ln_b, apply_affine)
    res = run_bass_kernel_spmd(nc, in_maps, list(range(N_CORES)))

    full = np.empty((B, NQ, LEN_K, D_MODEL), np.float32)
    for c in range(N_CORES):
        b, half = c // 2, c % 2
        full[b, :, 128 * half:128 * half + 128, :] = _decode_out(
            res.results[c]["out"])
    return full


# revision 17
# speedup vs baseline: 1.1354x; 1.1354x over previous
"""Trainium2 Bass kernel for nn_MultiHeadQuery_selfattention.

Sharding: 8 cores = 4 batches x 2 lk-halves. Core c handles batch b=c//2 and
key rows L = [128*(c%2), 128*(c%2)+128). Each core computes its output slice
out[b, :, L, :] fully independently (no collectives). A single NEFF serves
all cores: for the upper half, the host swaps the two lk-halves of k and v
(softmax/attention sums are permutation-invariant over keys), so on device
"local lk 0..127" is always the core's half.

Math per core (restructured from the reference):
  khT/vhT/qhT/quT: per-head projections kept transposed [head_dim, token]
  attn_h  = softmax(qh_h @ kh_h.T / 8)          rows = my 128 lk
  out_h   = attn_h @ vh_h                       [128, 64]
  outW    = concat_h(out_h) @ fc_w.T            [128, 512]
  base    = outW + q[b, L, :] + fc_b            [128, 512]
  vW_h    = vh_h[L] @ fc_w.T[64h:64h+64]        [128, 512]
  qattn_h = softmax(qu_h @ kh_h.T / 8)          [64, 256]
  res[nq, l, m] = base[l, m] + sum_h qattn[h, nq, l] * vW[h, l, m]
  out = layernorm(res) * ln_w + ln_b

res is produced on the PE as 128 small matmuls with K=9: stationary =
[qattn rows; ones], moving = [vW rows; base], so PSUM holds res directly.
lk pairs share one PSUM bank [128, 512] = (2 lk x 64 nq) rows; LN stats via
bn_stats/bn_aggr; normalize on ACT. QB/VB hold the K=9 operands in 4
partition groups (local lk l -> partitions 32*(l//32)+{0..8}, slot l%32).
"""

import numpy as np

N_HEAD, D_MODEL, D_K, D_V = 8, 512, 64, 64
B, LEN_K, NQ = 4, 256, 64
LN_EPS = 1e-5
N_CORES = 8

_CACHE = {}


def _build(apply_affine: bool):
    import contextlib

    import concourse.tile as tile
    from concourse import bacc, mybir

    f32 = mybir.dt.float32
    Alu = mybir.AluOpType
    Act = mybir.ActivationFunctionType

    nc = bacc.Bacc("TRN2", target_bir_lowering=False, debug=False,
                   num_devices=N_CORES)

    def din(name, shape):
        return nc.dram_tensor(name, shape, f32, kind="ExternalInput")

    # device-layout inputs ([512, N] host tensors stored as [128, 4, N])
    kT = din("kT", [128, 4, 256])
    vT = din("vT", [128, 4, 256])
    qT = din("qT", [128, 4, 128])     # only my 128 lq columns
    quT = din("quT", [128, 4, 64])
    wq = din("wq", [128, 4, 512])     # w_qs.T
    wk = din("wk", [128, 4, 512])
    wv = din("wv", [128, 4, 512])
    wu = din("wu", [128, 4, 512])
    fwT = din("fwT", [128, 4, 512])   # fc_w.T
    qpb = din("qpb", [128, 512])      # q[b, L, :] + fc_b
    ident = din("ident", [128, 128])
    onesd = din("ones", [32, 64])
    if apply_affine:
        lnw_r = din("lnw_r", [128, 512])
        lnb_r = din("lnb_r", [128, 512])
    # pass u, column-block g holds rows (lkk, nq) of lk = 32g + 2u + lkk
    outd = nc.dram_tensor("out", [16, 128, 2048], f32, kind="ExternalOutput")

    with tile.TileContext(nc) as tc, contextlib.ExitStack() as ctx:
        const = ctx.enter_context(tc.tile_pool(name="const", bufs=1))
        stage = ctx.enter_context(tc.tile_pool(name="stage", bufs=4))
        small = ctx.enter_context(tc.tile_pool(name="small", bufs=8))
        pp_cm = tc.tile_pool(name="pp", bufs=8, space="PSUM")
        pp = pp_cm.__enter__()

        def load(dram, shape):
            t = const.tile(shape, f32, tag=dram.name)
            nc.sync.dma_start(t[:], dram[:])
            return t

        kT_s = load(kT, [128, 4, 256])
        vT_s = load(vT, [128, 4, 256])
        qT_s = load(qT, [128, 4, 128])
        quT_s = load(quT, [128, 4, 64])
        wq_s = load(wq, [128, 4, 512])
        wk_s = load(wk, [128, 4, 512])
        wv_s = load(wv, [128, 4, 512])
        wu_s = load(wu, [128, 4, 512])
        fwT_s = load(fwT, [128, 4, 512])
        qpb_s = load(qpb, [128, 512])
        id_s = load(ident, [128, 128])
        if apply_affine:
            lnw_s = load(lnw_r, [128, 512])
            lnb_s = load(lnb_r, [128, 512])

        # ---- projections: xhT[hd, tok] = sum_dm w.T[dm, hd] * x.T[dm, tok]
        def project(w_s, x_s, ncols, tag):
            dst = const.tile([128, 4, ncols], f32, tag=tag)
            for mo in range(4):
                ps = pp.tile([128, ncols], f32, tag="ps")
                for ki in range(4):
                    nc.tensor.matmul(
                        ps[:], w_s[:, ki, mo * 128:(mo + 1) * 128],
                        x_s[:, ki, :], start=(ki == 0), stop=(ki == 3))
                nc.scalar.copy(dst[:, mo, :], ps[:])
            return dst

        khT = project(wk_s, kT_s, 256, "khT")
        vhT = project(wv_s, vT_s, 256, "vhT")
        qhT = project(wq_s, qT_s, 128, "qhT")
        quhT = project(wu_s, quT_s, 64, "quhT")

        # ---- vh untransposed [lk, hd] as two [128, 512] tiles
        vh = [const.tile([128, 512], f32, name=f"vh{j}", tag=f"vh{j}")
              for j in range(2)]
        for lj in range(2):
            for hc in range(4):
                ps = pp.tile([128, 128], f32, tag="ps")
                nc.tensor.transpose(
                    ps[:], vhT[:, hc, lj * 128:(lj + 1) * 128], id_s[:])
                nc.scalar.copy(vh[lj][:, hc * 128:(hc + 1) * 128], ps[:])

        # ---- per-head attention
        outT = const.tile([128, 4, 128], f32, tag="outT")  # [hd, lq]
        eps_t = const.tile([128, 1], f32, tag="eps_t")
        nc.vector.memset(eps_t[:], LN_EPS)
        VB = const.tile([128, 32, 512], f32, tag="VB")
        QB = const.tile([128, 32, 64], f32, tag="QB")
        for g in range(4):
            nc.sync.dma_start(QB[32 * g + 8: 32 * g + 9, :, :], onesd[:])

        def hslice(t, h, cols):
            po = (h % 2) * 64
            return t[po:po + 64, h // 2, cols] if cols is not None else \
                t[po:po + 64, h // 2, :]

        # ---- vW_h -> VB rows h (independent of attention; DMAs start early)
        for h in range(8):
            vw_ps = pp.tile([128, 512], f32, tag="ps")
            nc.tensor.matmul(vw_ps[:], hslice(vhT, h, slice(0, 128)),
                             hslice(fwT_s, h, None), start=True, stop=True)
            vw_sb = stage.tile([128, 512], f32, tag="vw_sb", bufs=4)
            nc.scalar.copy(vw_sb[:], vw_ps[:])
            for g in range(4):
                nc.sync.dma_start(VB[32 * g + h: 32 * g + h + 1, :, :],
                                  vw_sb[32 * g: 32 * g + 32, :])

        # ---- qattn_h -> QB rows h
        qaexps = []
        for h in range(8):
            ps_q = pp.tile([64, 256], f32, tag="ps")
            nc.tensor.matmul(ps_q[:], hslice(quhT, h, None),
                             hslice(khT, h, None), start=True, stop=True)
            qnmax8 = small.tile([64, 1], f32, tag="qnmax8")
            nc.vector.tensor_reduce(qnmax8[:], ps_q[:], mybir.AxisListType.X,
                                    Alu.max, negate=True)
            nc.vector.tensor_scalar_mul(qnmax8[:], qnmax8[:], 0.125)
            qaexp = stage.tile([64, 256], f32, tag="qaexp", bufs=8)
            qrsum = small.tile([64, 1], f32, tag="qrsum")
            nc.scalar.activation(qaexp[:], ps_q[:], Act.Exp,
                                 bias=qnmax8[:], scale=0.125,
                                 accum_out=qrsum[:])
            qrrec = small.tile([64, 1], f32, tag="qrrec")
            nc.vector.reciprocal(qrrec[:], qrsum[:])
            nc.vector.tensor_scalar_mul(qaexp[:], qaexp[:], qrrec[:])
            qaexps.append(qaexp)
        for h in range(8):
            qaT_ps = pp.tile([128, 64], f32, tag="ps")
            nc.tensor.transpose(qaT_ps[:], qaexps[h][:, 0:128],
                                id_s[:64, :64])
            qa_sb = stage.tile([128, 64], f32, tag="qa_sb", bufs=4)
            nc.vector.tensor_copy(qa_sb[:], qaT_ps[:])
            for g in range(4):
                nc.sync.dma_start(QB[32 * g + h: 32 * g + h + 1, :, :],
                                  qa_sb[32 * g: 32 * g + 32, :])

        # ---- attention, stage-major across heads
        aexps, rrecs = [], []
        for h in range(8):
            ps_s = pp.tile([128, 256], f32, tag="ps")
            nc.tensor.matmul(ps_s[:], hslice(qhT, h, None),
                             hslice(khT, h, None), start=True, stop=True)
            nmax8 = small.tile([128, 1], f32, tag="nmax8")
            nc.vector.tensor_reduce(nmax8[:], ps_s[:], mybir.AxisListType.X,
                                    Alu.max, negate=True)
            nc.vector.tensor_scalar_mul(nmax8[:], nmax8[:], 0.125)
            aexp = stage.tile([128, 256], f32, tag="aexp", bufs=8)
            rsum = small.tile([128, 1], f32, tag="rsum")
            nc.scalar.activation(aexp[:], ps_s[:], Act.Exp,
                                 bias=nmax8[:], scale=0.125, accum_out=rsum[:])
            rrec = small.tile([128, 1], f32, tag="rrec")
            nc.vector.reciprocal(rrec[:], rsum[:])
            aexps.append(aexp)
            rrecs.append(rrec)
        aTs = []
        for h in range(8):
            pair = []
            for lj in range(2):
                aT_ps = pp.tile([128, 128], f32, tag="ps")
                nc.tensor.transpose(
                    aT_ps[:], aexps[h][:, lj * 128:(lj + 1) * 128], id_s[:])
                aT_sb = stage.tile([128, 128], f32, tag="aT", bufs=10)
                nc.scalar.copy(aT_sb[:], aT_ps[:])
                pair.append(aT_sb)
            aTs.append(pair)
        outns = []
        for h in range(8):
            out_ps = pp.tile([128, 64], f32, tag="ps")
            for lj in range(2):
                nc.tensor.matmul(out_ps[:], aTs[h][lj][:],
                                 vh[lj][:, h * 64:(h + 1) * 64],
                                 start=(lj == 0), stop=(lj == 1))
            outn = stage.tile([128, 64], f32, tag="outn", bufs=8)
            nc.vector.tensor_scalar_mul(outn[:], out_ps[:], rrecs[h][:])
            outns.append(outn)
        for h in range(8):
            oT_ps = pp.tile([64, 128], f32, tag="ps")
            nc.tensor.transpose(oT_ps[:], outns[h][:], id_s[:])
            nc.scalar.copy(outT[(h % 2) * 64:(h % 2) * 64 + 64, h // 2, :],
                           oT_ps[:])

        # ---- outW + base
        ow_ps = pp.tile([128, 512], f32, tag="ps")
        for c in range(4):
            nc.tensor.matmul(ow_ps[:], outT[:, c, :], fwT_s[:, c, :],
                             start=(c == 0), stop=(c == 3))
        base = const.tile([128, 512], f32, tag="base")
        nc.vector.tensor_add(base[:], ow_ps[:], qpb_s[:])
        for g in range(4):
            nc.sync.dma_start(VB[32 * g + 8: 32 * g + 9, :, :],
                              base[32 * g: 32 * g + 32, :])

        # ---- termB + layernorm: 16 passes, 8 lk each (4 row x 2 col tiles)
        pp_cm.__exit__(None, None, None)
        with tc.tile_pool(name="pb", bufs=8, space="PSUM") as pb:
            for u in range(16):
                banks = []
                for g in range(4):
                    bank = pb.tile([128, 512], f32, name=f"bank{u}_{g}",
                                   tag="bank")
                    banks.append(bank)
                    for c in range(2):
                        s = 2 * u + c
                        nc.tensor.matmul(
                            bank[64 * c: 64 * c + 64, :],
                            QB[32 * g: 32 * g + 9, s, :],
                            VB[32 * g: 32 * g + 9, s, :],
                            start=True, stop=True,
                            tile_position=(32 * g, 64 * c))
                st6 = small.tile([128, 6, 4], f32, tag="st6")
                st2 = small.tile([128, 2, 4], f32, tag="st2")
                for g in range(4):
                    nc.vector.bn_stats(st6[:, :, g:g + 1], banks[g][:])
                    nc.vector.bn_aggr(st2[:, :, g:g + 1], st6[:, :, g:g + 1])
                std4 = small.tile([128, 4], f32, tag="std4")
                nc.scalar.activation(std4[:], st2[:, 1, :], Act.Sqrt,
                                     bias=eps_t[:], scale=1.0)
                rstd4 = small.tile([128, 4], f32, tag="rstd4")
                nc.vector.reciprocal(rstd4[:], std4[:])
                nmr4 = small.tile([128, 4], f32, tag="nmr4")
                nc.vector.scalar_tensor_tensor(nmr4[:], st2[:, 0, :], -1.0,
                                               rstd4[:], op0=Alu.mult,
                                               op1=Alu.mult)
                big = stage.tile([128, 2048], f32, tag="big", bufs=3)
                for g in range(4):
                    nc.scalar.activation(big[:, 512 * g: 512 * g + 512],
                                         banks[g][:], Act.Identity,
                                         bias=nmr4[:, g:g + 1],
                                         scale=rstd4[:, g:g + 1])
                    if apply_affine:
                        nc.vector.tensor_mul(big[:, 512 * g: 512 * g + 512],
                                             big[:, 512 * g: 512 * g + 512],
                                             lnw_s[:])
                        nc.vector.tensor_add(big[:, 512 * g: 512 * g + 512],
                                             big[:, 512 * g: 512 * g + 512],
                                             lnb_s[:])
                nc.sync.dma_start(outd[u], big[:])

    nc.compile()
    return nc


def _get_nc(apply_affine: bool):
    if apply_affine not in _CACHE:
        _CACHE[apply_affine] = _build(apply_affine)
    return _CACHE[apply_affine]


def _decode_out(raw):
    # [16, 128, 2048] -> [nq, local lk, m]
    arr = np.asarray(raw).reshape(16, 2, 64, 4, 512)
    return arr.transpose(2, 3, 0, 1, 4).reshape(NQ, 128, 512)


def _to_dev(x):
    # [512, N] -> [128, 4, N]
    return np.ascontiguousarray(
        x.reshape(4, 128, x.shape[1]).transpose(1, 0, 2))


def make_in_maps(q, k, v, query, w_qs, w_ks, w_vs, w_query, fc_w, fc_b,
                 ln_w, ln_b, apply_affine):
    ident = np.eye(128, dtype=np.float32)
    wqT = _to_dev(w_qs.T)
    wkT = _to_dev(w_ks.T)
    wvT = _to_dev(w_vs.T)
    wuT = _to_dev(w_query.T)
    fwT = _to_dev(fc_w.T)
    in_maps = []
    for c in range(N_CORES):
        b, half = c // 2, c % 2
        L0 = 128 * half
        # swap lk halves so that local lk 0..127 is always this core's half
        perm = np.r_[L0:L0 + 128, (128 - L0):(128 - L0) + 128]
        m = {
            "kT": _to_dev(np.ascontiguousarray(k[b][perm].T)),
            "vT": _to_dev(np.ascontiguousarray(v[b][perm].T)),
            "qT": _to_dev(np.ascontiguousarray(q[b, L0:L0 + 128].T)),
            "quT": _to_dev(query[b].T),
            "wq": wqT, "wk": wkT, "wv": wvT, "wu": wuT, "fwT": fwT,
            "qpb": q[b, L0:L0 + 128] + fc_b,
            "ident": ident,
            "ones": np.ones((32, 64), np.float32),
        }
        if apply_affine:
            m["lnw_r"] = np.ascontiguousarray(
                np.broadcast_to(ln_w, (128, 512)))
            m["lnb_r"] = np.ascontiguousarray(
                np.broadcast_to(ln_b, (128, 512)))
        in_maps.append(m)
    return in_maps


def kernel(q, k, v, query, w_qs, w_ks, w_vs, w_query, fc_w, fc_b, ln_w, ln_b):
    from concourse.bass_utils import run_bass_kernel_spmd

    q, k, v, query = (np.asarray(a, np.float32) for a in (q, k, v, query))
    w_qs, w_ks, w_vs, w_query, fc_w = (
        np.asarray(a, np.float32) for a in (w_qs, w_ks, w_vs, w_query, fc_w))
    fc_b, ln_w, ln_b = (np.asarray(a, np.float32)
                        for a in (fc_b, ln_w, ln_b))

    apply_affine = not (np.all(ln_w == 1.0) and np.all(ln_b == 0.0))
    nc = _get_nc(apply_affine)
    in_maps = make_in_maps(q, k, v, query, w_qs, w_ks, w_vs, w_query,
                           fc_w, fc_b, ln_w, ln_b, apply_affine)
    res = run_bass_kernel_spmd(nc, in_maps, list(range(N_CORES)))

    full = np.empty((B, NQ, LEN_K, D_MODEL), np.float32)
    for c in range(N_CORES):
        b, half = c // 2, c % 2
        full[b, :, 128 * half:128 * half + 128, :] = _decode_out(
            res.results[c]["out"])
    return full


# revision 25
# speedup vs baseline: 1.1497x; 1.0126x over previous
"""Trainium2 Bass kernel for nn_MultiHeadQuery_selfattention.

Sharding: 8 cores = 4 batches x 2 lk-halves. Core c handles batch b=c//2 and
key rows L = [128*(c%2), 128*(c%2)+128). Each core computes its output slice
out[b, :, L, :] fully independently (no collectives). A single NEFF serves
all cores: for the upper half, the host swaps the two lk-halves of k and v
(softmax/attention sums are permutation-invariant over keys), so on device
"local lk 0..127" is always the core's half.

Math per core (restructured from the reference):
  khT/vhT/qhT/quT: per-head projections kept transposed [head_dim, token]
  attn_h  = softmax(qh_h @ kh_h.T / 8)          rows = my 128 lk
  out_h   = attn_h @ vh_h                       [128, 64]
  outW    = concat_h(out_h) @ fc_w.T            [128, 512]
  base    = outW + q[b, L, :] + fc_b            [128, 512]
  vW_h    = vh_h[L] @ fc_w.T[64h:64h+64]        [128, 512]
  qattn_h = softmax(qu_h @ kh_h.T / 8)          [64, 256]
  res[nq, l, m] = base[l, m] + sum_h qattn[h, nq, l] * vW[h, l, m]
  out = layernorm(res) * ln_w + ln_b

res is produced on the PE as 128 small matmuls with K=9: stationary =
[qattn rows; ones], moving = [vW rows; base], so PSUM holds res directly.
lk pairs share one PSUM bank [128, 512] = (2 lk x 64 nq) rows; LN stats via
bn_stats/bn_aggr; normalize on ACT. QB/VB hold the K=9 operands in 4
partition groups (local lk l -> partitions 32*(l//32)+{0..8}, slot l%32).
"""

import numpy as np

N_HEAD, D_MODEL, D_K, D_V = 8, 512, 64, 64
B, LEN_K, NQ = 4, 256, 64
LN_EPS = 1e-5
N_CORES = 8

_CACHE = {}


def _build(apply_affine: bool):
    import contextlib

    import concourse.tile as tile
    from concourse import bacc, mybir

    f32 = mybir.dt.float32
    Alu = mybir.AluOpType
    Act = mybir.ActivationFunctionType

    nc = bacc.Bacc("TRN2", target_bir_lowering=False, debug=False,
                   num_devices=N_CORES)

    def din(name, shape):
        return nc.dram_tensor(name, shape, f32, kind="ExternalInput")

    # device-layout inputs ([512, N] host tensors stored as [128, 4, N])
    kT = din("kT", [128, 4, 256])
    vT = din("vT", [128, 4, 256])
    qT = din("qT", [128, 4, 128])     # only my 128 lq columns
    quT = din("quT", [128, 4, 64])
    wq = din("wq", [128, 4, 512])     # w_qs.T
    wk = din("wk", [128, 4, 512])
    wv = din("wv", [128, 4, 512])
    wu = din("wu", [128, 4, 512])
    fwT = din("fwT", [128, 4, 512])   # fc_w.T
    qpb = din("qpb", [128, 512])      # q[b, L, :] + fc_b
    ident = din("ident", [128, 128])
    onesd = din("ones", [4, 32, 64])
    if apply_affine:
        lnw_r = din("lnw_r", [128, 512])
        lnb_r = din("lnb_r", [128, 512])
    # pass u, column-block g holds rows (lkk, nq) of lk = 32g + 2u + lkk
    outd = nc.dram_tensor("out", [16, 128, 2048], f32, kind="ExternalOutput")

    with tile.TileContext(nc) as tc, contextlib.ExitStack() as ctx:
        const = ctx.enter_context(tc.tile_pool(name="const", bufs=1))
        stage = ctx.enter_context(tc.tile_pool(name="stage", bufs=4))
        small = ctx.enter_context(tc.tile_pool(name="small", bufs=8))
        pp_cm = tc.tile_pool(name="pp", bufs=8, space="PSUM")
        pp = pp_cm.__enter__()

        def load(dram, shape):
            t = const.tile(shape, f32, tag=dram.name)
            nc.sync.dma_start(t[:], dram[:])
            return t

        kT_s = load(kT, [128, 4, 256])
        vT_s = load(vT, [128, 4, 256])
        qT_s = load(qT, [128, 4, 128])
        quT_s = load(quT, [128, 4, 64])
        wq_s = load(wq, [128, 4, 512])
        wk_s = load(wk, [128, 4, 512])
        wv_s = load(wv, [128, 4, 512])
        wu_s = load(wu, [128, 4, 512])
        fwT_s = load(fwT, [128, 4, 512])
        qpb_s = load(qpb, [128, 512])
        id_s = load(ident, [128, 128])
        if apply_affine:
            lnw_s = load(lnw_r, [128, 512])
            lnb_s = load(lnb_r, [128, 512])

        # ---- projections: xhT[hd, tok] = sum_dm w.T[dm, hd] * x.T[dm, tok]
        def project(w_s, x_s, ncols, tag):
            dst = const.tile([128, 4, ncols], f32, tag=tag)
            for mo in range(4):
                ps = pp.tile([128, ncols], f32, tag="ps")
                for ki in range(4):
                    nc.tensor.matmul(
                        ps[:], w_s[:, ki, mo * 128:(mo + 1) * 128],
                        x_s[:, ki, :], start=(ki == 0), stop=(ki == 3))
                nc.scalar.copy(dst[:, mo, :], ps[:])
            return dst

        khT = project(wk_s, kT_s, 256, "khT")
        vhT = project(wv_s, vT_s, 256, "vhT")
        qhT = project(wq_s, qT_s, 128, "qhT")
        quhT = project(wu_s, quT_s, 64, "quhT")

        # ---- vh untransposed [lk, hd] as two [128, 512] tiles
        vh = [const.tile([128, 512], f32, name=f"vh{j}", tag=f"vh{j}")
              for j in range(2)]
        for lj in range(2):
            for hc in range(4):
                ps = pp.tile([128, 128], f32, tag="ps")
                nc.tensor.transpose(
                    ps[:], vhT[:, hc, lj * 128:(lj + 1) * 128], id_s[:])
                nc.scalar.copy(vh[lj][:, hc * 128:(hc + 1) * 128], ps[:])

        # ---- per-head attention
        outT = const.tile([128, 4, 128], f32, tag="outT")  # [hd, lq]
        eps_t = const.tile([128, 1], f32, tag="eps_t")
        nc.vector.memset(eps_t[:], LN_EPS)
        VB = const.tile([128, 32, 512], f32, tag="VB")
        QB = const.tile([128, 32, 64], f32, tag="QB")
        for g in range(4):
            nc.gpsimd.dma_start(QB[32 * g + 8: 32 * g + 9, :, :],
                                onesd[g])

        def hslice(t, h, cols):
            po = (h % 2) * 64
            return t[po:po + 64, h // 2, cols] if cols is not None else \
                t[po:po + 64, h // 2, :]

        # ---- vW_h -> VB rows h (independent of attention; DMAs start early)
        for h in range(8):
            vw_ps = pp.tile([128, 512], f32, tag="ps")
            nc.tensor.matmul(vw_ps[:], hslice(vhT, h, slice(0, 128)),
                             hslice(fwT_s, h, None), start=True, stop=True)
            vw_sb = stage.tile([128, 512], f32, tag="vw_sb", bufs=4)
            nc.scalar.copy(vw_sb[:], vw_ps[:])
            for g in range(4):
                nc.gpsimd.dma_start(VB[32 * g + h: 32 * g + h + 1, :, :],
                                    vw_sb[32 * g: 32 * g + 32, :])

        # ---- qattn scores + softmax (transposes emitted later)
        qaexps = []
        for h in range(8):
            ps_q = pp.tile([64, 256], f32, tag="ps")
            nc.tensor.matmul(ps_q[:], hslice(quhT, h, None),
                             hslice(khT, h, None), start=True, stop=True)
            qnmax8 = small.tile([64, 1], f32, tag="qnmax8")
            nc.vector.tensor_reduce(qnmax8[:], ps_q[:], mybir.AxisListType.X,
                                    Alu.max, negate=True)
            nc.vector.tensor_scalar_mul(qnmax8[:], qnmax8[:], 0.125)
            qaexp = stage.tile([64, 256], f32, tag="qaexp", bufs=8)
            qrsum = small.tile([64, 1], f32, tag="qrsum")
            nc.scalar.activation(qaexp[:], ps_q[:], Act.Exp,
                                 bias=qnmax8[:], scale=0.125,
                                 accum_out=qrsum[:])
            qrrec = small.tile([64, 1], f32, tag="qrrec")
            nc.vector.reciprocal(qrrec[:], qrsum[:])
            nc.vector.tensor_scalar_mul(qaexp[:], qaexp[:], qrrec[:])
            qaexps.append(qaexp)

        # ---- attention scores + softmax
        aexps, rrecs = [], []
        for h in range(8):
            ps_s = pp.tile([128, 256], f32, tag="ps")
            nc.tensor.matmul(ps_s[:], hslice(qhT, h, None),
                             hslice(khT, h, None), start=True, stop=True)
            nmax8 = small.tile([128, 1], f32, tag="nmax8")
            nc.vector.tensor_reduce(nmax8[:], ps_s[:], mybir.AxisListType.X,
                                    Alu.max, negate=True)
            nc.vector.tensor_scalar_mul(nmax8[:], nmax8[:], 0.125)
            aexp = stage.tile([128, 256], f32, tag="aexp", bufs=8)
            rsum = small.tile([128, 1], f32, tag="rsum")
            nc.scalar.activation(aexp[:], ps_s[:], Act.Exp,
                                 bias=nmax8[:], scale=0.125, accum_out=rsum[:])
            rrec = small.tile([128, 1], f32, tag="rrec")
            nc.vector.reciprocal(rrec[:], rsum[:])
            aexps.append(aexp)
            rrecs.append(rrec)

        # ---- qattn transposes -> QB (one strided DMA per head)
        for h in range(8):
            qaT_ps = pp.tile([128, 64], f32, tag="ps")
            nc.tensor.transpose(qaT_ps[:], qaexps[h][:, 0:128],
                                id_s[:64, :64])
            qa_sb = stage.tile([128, 64], f32, tag="qa_sb", bufs=4)
            nc.scalar.copy(qa_sb[:], qaT_ps[:])
            for g in range(4):
                nc.gpsimd.dma_start(QB[32 * g + h: 32 * g + h + 1, :, :],
                                    qa_sb[32 * g: 32 * g + 32, :])

        aTs = []
        for h in range(8):
            pair = []
            for lj in range(2):
                aT_ps = pp.tile([128, 128], f32, tag="ps")
                nc.tensor.transpose(
                    aT_ps[:], aexps[h][:, lj * 128:(lj + 1) * 128], id_s[:])
                aT_sb = stage.tile([128, 128], f32, tag="aT", bufs=10)
                nc.scalar.copy(aT_sb[:], aT_ps[:])
                pair.append(aT_sb)
            aTs.append(pair)
        outns = []
        for h in range(8):
            out_ps = pp.tile([128, 64], f32, tag="ps")
            for lj in range(2):
                nc.tensor.matmul(out_ps[:], aTs[h][lj][:],
                                 vh[lj][:, h * 64:(h + 1) * 64],
                                 start=(lj == 0), stop=(lj == 1))
            outn = stage.tile([128, 64], f32, tag="outn", bufs=8)
            nc.vector.tensor_scalar_mul(outn[:], out_ps[:], rrecs[h][:])
            outns.append(outn)
        for h in range(8):
            oT_ps = pp.tile([64, 128], f32, tag="ps")
            nc.tensor.transpose(oT_ps[:], outns[h][:], id_s[:])
            nc.scalar.copy(outT[(h % 2) * 64:(h % 2) * 64 + 64, h // 2, :],
                           oT_ps[:])

        # ---- outW + base
        ow_ps = pp.tile([128, 512], f32, tag="ps")
        for c in range(4):
            nc.tensor.matmul(ow_ps[:], outT[:, c, :], fwT_s[:, c, :],
                             start=(c == 0), stop=(c == 3))
        base = const.tile([128, 512], f32, tag="base")
        nc.vector.tensor_add(base[:], ow_ps[:], qpb_s[:])
        for g in range(4):
            nc.gpsimd.dma_start(VB[32 * g + 8: 32 * g + 9, :, :],
                                base[32 * g: 32 * g + 32, :])

        # ---- termB + layernorm: 16 passes, 8 lk each (4 row x 2 col tiles)
        pp_cm.__exit__(None, None, None)
        with tc.tile_pool(name="pb", bufs=8, space="PSUM") as pb:
            for u in range(16):
                banks = []
                for g in range(4):
                    bank = pb.tile([128, 512], f32, name=f"bank{u}_{g}",
                                   tag="bank")
                    banks.append(bank)
                    for c in range(2):
                        s = 2 * u + c
                        nc.tensor.matmul(
                            bank[64 * c: 64 * c + 64, :],
                            QB[32 * g: 32 * g + 9, s, :],
                            VB[32 * g: 32 * g + 9, s, :],
                            start=True, stop=True,
                            tile_position=(32 * g, 64 * c))
                st6 = small.tile([128, 6, 4], f32, tag="st6")
                st2 = small.tile([128, 2, 4], f32, tag="st2")
                for g in range(4):
                    nc.vector.bn_stats(st6[:, :, g:g + 1], banks[g][:])
                    nc.vector.bn_aggr(st2[:, :, g:g + 1], st6[:, :, g:g + 1])
                std4 = small.tile([128, 4], f32, tag="std4")
                nc.scalar.activation(std4[:], st2[:, 1, :], Act.Sqrt,
                                     bias=eps_t[:], scale=1.0)
                rstd4 = small.tile([128, 4], f32, tag="rstd4")
                nc.vector.reciprocal(rstd4[:], std4[:])
                nmr4 = small.tile([128, 4], f32, tag="nmr4")
                nc.vector.scalar_tensor_tensor(nmr4[:], st2[:, 0, :], -1.0,
                                               rstd4[:], op0=Alu.mult,
                                               op1=Alu.mult)
                big = stage.tile([128, 2048], f32, tag="big", bufs=2)
                for g in range(4):
                    nc.scalar.activation(big[:, 512 * g: 512 * g + 512],
                                         banks[g][:], Act.Identity,
                                         bias=nmr4[:, g:g + 1],
                                         scale=rstd4[:, g:g + 1])
                    if apply_affine:
                        nc.vector.tensor_mul(big[:, 512 * g: 512 * g + 512],
                                             big[:, 512 * g: 512 * g + 512],
                                             lnw_s[:])
                        nc.vector.tensor_add(big[:, 512 * g: 512 * g + 512],
                                             big[:, 512 * g: 512 * g + 512],
                                             lnb_s[:])
                nc.sync.dma_start(outd[u], big[:])

    nc.compile()
    return nc


def _get_nc(apply_affine: bool):
    if apply_affine not in _CACHE:
        _CACHE[apply_affine] = _build(apply_affine)
    return _CACHE[apply_affine]


def _decode_out(raw):
    # [16, 128, 2048] -> [nq, local lk, m]
    arr = np.asarray(raw).reshape(16, 2, 64, 4, 512)
    return arr.transpose(2, 3, 0, 1, 4).reshape(NQ, 128, 512)


def _to_dev(x):
    # [512, N] -> [128, 4, N]
    return np.ascontiguousarray(
        x.reshape(4, 128, x.shape[1]).transpose(1, 0, 2))


def make_in_maps(q, k, v, query, w_qs, w_ks, w_vs, w_query, fc_w, fc_b,
                 ln_w, ln_b, apply_affine):
    ident = np.eye(128, dtype=np.float32)
    wqT = _to_dev(w_qs.T)
    wkT = _to_dev(w_ks.T)
    wvT = _to_dev(w_vs.T)
    wuT = _to_dev(w_query.T)
    fwT = _to_dev(fc_w.T)
    in_maps = []
    for c in range(N_CORES):
        b, half = c // 2, c % 2
        L0 = 128 * half
        # swap lk halves so that local lk 0..127 is always this core's half
        perm = np.r_[L0:L0 + 128, (128 - L0):(128 - L0) + 128]
        m = {
            "kT": _to_dev(np.ascontiguousarray(k[b][perm].T)),
            "vT": _to_dev(np.ascontiguousarray(v[b][perm].T)),
            "qT": _to_dev(np.ascontiguousarray(q[b, L0:L0 + 128].T)),
            "quT": _to_dev(query[b].T),
            "wq": wqT, "wk": wkT, "wv": wvT, "wu": wuT, "fwT": fwT,
            "qpb": q[b, L0:L0 + 128] + fc_b,
            "ident": ident,
            "ones": np.ones((4, 32, 64), np.float32),
        }
        if apply_affine:
            m["lnw_r"] = np.ascontiguousarray(
                np.broadcast_to(ln_w, (128, 512)))
            m["lnb_r"] = np.ascontiguousarray(
                np.broadcast_to(ln_b, (128, 512)))
        in_maps.append(m)
    return in_maps


def kernel(q, k, v, query, w_qs, w_ks, w_vs, w_query, fc_w, fc_b, ln_w, ln_b):
    from concourse.bass_utils import run_bass_kernel_spmd

    q, k, v, query = (np.asarray(a, np.float32) for a in (q, k, v, query))
    w_qs, w_ks, w_vs, w_query, fc_w = (
        np.asarray(a, np.float32) for a in (w_qs, w_ks, w_vs, w_query, fc_w))
    fc_b, ln_w, ln_b = (np.asarray(a, np.float32)
                        for a in (fc_b, ln_w, ln_b))

    apply_affine = not (np.all(ln_w == 1.0) and np.all(ln_b == 0.0))
    nc = _get_nc(apply_affine)
    in_maps = make_in_maps(q, k, v, query, w_qs, w_ks, w_vs, w_query,
                           fc_w, fc_b, ln_w, ln_b, apply_affine)
    res = run_bass_kernel_spmd(nc, in_maps, list(range(N_CORES)))

    full = np.empty((B, NQ, LEN_K, D_MODEL), np.float32)
    for c in range(N_CORES):
        b, half = c // 2, c % 2
        full[b, :, 128 * half:128 * half + 128, :] = _decode_out(
            res.results[c]["out"])
    return full


# revision 26
# speedup vs baseline: 1.1849x; 1.0306x over previous
"""Trainium2 Bass kernel for nn_MultiHeadQuery_selfattention.

Sharding: 8 cores = 4 batches x 2 lk-halves. Core c handles batch b=c//2 and
key rows L = [128*(c%2), 128*(c%2)+128). Each core computes its output slice
out[b, :, L, :] fully independently (no collectives). A single NEFF serves
all cores: for the upper half, the host swaps the two lk-halves of k and v
(softmax/attention sums are permutation-invariant over keys), so on device
"local lk 0..127" is always the core's half.

Math per core (restructured from the reference):
  khT/vhT/qhT/quT: per-head projections kept transposed [head_dim, token]
  attn_h  = softmax(qh_h @ kh_h.T / 8)          rows = my 128 lk
  out_h   = attn_h @ vh_h                       [128, 64]
  outW    = concat_h(out_h) @ fc_w.T            [128, 512]
  base    = outW + q[b, L, :] + fc_b            [128, 512]
  vW_h    = vh_h[L] @ fc_w.T[64h:64h+64]        [128, 512]
  qattn_h = softmax(qu_h @ kh_h.T / 8)          [64, 256]
  res[nq, l, m] = base[l, m] + sum_h qattn[h, nq, l] * vW[h, l, m]
  out = layernorm(res) * ln_w + ln_b

res is produced on the PE as 128 small matmuls with K=9: stationary =
[qattn rows; ones], moving = [vW rows; base], so PSUM holds res directly.
lk pairs share one PSUM bank [128, 512] = (2 lk x 64 nq) rows; LN stats via
bn_stats/bn_aggr; normalize on ACT. QB/VB hold the K=9 operands in 4
partition groups (local lk l -> partitions 32*(l//32)+{0..8}, slot l%32).
"""

import numpy as np

N_HEAD, D_MODEL, D_K, D_V = 8, 512, 64, 64
B, LEN_K, NQ = 4, 256, 64
LN_EPS = 1e-5
N_CORES = 8

_CACHE = {}


def _build(apply_affine: bool):
    import contextlib

    import concourse.tile as tile
    from concourse import bacc, mybir

    f32 = mybir.dt.float32
    Alu = mybir.AluOpType
    Act = mybir.ActivationFunctionType

    nc = bacc.Bacc("TRN2", target_bir_lowering=False, debug=False,
                   num_devices=N_CORES)

    def din(name, shape):
        return nc.dram_tensor(name, shape, f32, kind="ExternalInput")

    # device-layout inputs ([512, N] host tensors stored as [128, 4, N])
    kT = din("kT", [128, 4, 256])
    vT = din("vT", [128, 4, 256])
    qT = din("qT", [128, 4, 128])     # only my 128 lq columns
    quT = din("quT", [128, 4, 64])
    wq = din("wq", [128, 4, 512])     # w_qs.T
    wk = din("wk", [128, 4, 512])
    wv = din("wv", [128, 4, 512])
    wu = din("wu", [128, 4, 512])
    fwT = din("fwT", [128, 4, 512])   # fc_w.T
    qpb = din("qpb", [128, 512])      # q[b, L, :] + fc_b
    ident = din("ident", [128, 128])
    onesd = din("ones", [4, 32, 64])
    if apply_affine:
        lnw_r = din("lnw_r", [128, 512])
        lnb_r = din("lnb_r", [128, 512])
    # pass u, column-block g holds rows (lkk, nq) of lk = 32g + 2u + lkk
    outd = nc.dram_tensor("out", [16, 128, 2048], f32, kind="ExternalOutput")

    with tile.TileContext(nc) as tc, contextlib.ExitStack() as ctx:
        const = ctx.enter_context(tc.tile_pool(name="const", bufs=1))
        stage = ctx.enter_context(tc.tile_pool(name="stage", bufs=4))
        small = ctx.enter_context(tc.tile_pool(name="small", bufs=8))
        pp_cm = tc.tile_pool(name="pp", bufs=8, space="PSUM")
        pp = pp_cm.__enter__()

        def load(dram, shape):
            t = const.tile(shape, f32, tag=dram.name)
            nc.sync.dma_start(t[:], dram[:])
            return t

        kT_s = load(kT, [128, 4, 256])
        vT_s = load(vT, [128, 4, 256])
        qT_s = load(qT, [128, 4, 128])
        quT_s = load(quT, [128, 4, 64])
        wq_s = load(wq, [128, 4, 512])
        wk_s = load(wk, [128, 4, 512])
        wv_s = load(wv, [128, 4, 512])
        wu_s = load(wu, [128, 4, 512])
        fwT_s = load(fwT, [128, 4, 512])
        qpb_s = load(qpb, [128, 512])
        id_s = load(ident, [128, 128])
        if apply_affine:
            lnw_s = load(lnw_r, [128, 512])
            lnb_s = load(lnb_r, [128, 512])

        # ---- projections: xhT[hd, tok] = sum_dm w.T[dm, hd] * x.T[dm, tok]
        def project(w_s, x_s, ncols, tag):
            dst = const.tile([128, 4, ncols], f32, tag=tag)
            for mo in range(4):
                ps = pp.tile([128, ncols], f32, tag="ps")
                for ki in range(4):
                    nc.tensor.matmul(
                        ps[:], w_s[:, ki, mo * 128:(mo + 1) * 128],
                        x_s[:, ki, :], start=(ki == 0), stop=(ki == 3))
                nc.scalar.copy(dst[:, mo, :], ps[:])
            return dst

        khT = project(wk_s, kT_s, 256, "khT")
        vhT = project(wv_s, vT_s, 256, "vhT")
        qhT = project(wq_s, qT_s, 128, "qhT")
        quhT = project(wu_s, quT_s, 64, "quhT")

        # ---- vh untransposed [lk, hd] as two [128, 512] tiles
        vh = [const.tile([128, 512], f32, name=f"vh{j}", tag=f"vh{j}")
              for j in range(2)]
        for lj in range(2):
            for hc in range(4):
                ps = pp.tile([128, 128], f32, tag="ps")
                nc.tensor.transpose(
                    ps[:], vhT[:, hc, lj * 128:(lj + 1) * 128], id_s[:])
                nc.scalar.copy(vh[lj][:, hc * 128:(hc + 1) * 128], ps[:])

        # ---- per-head attention
        outT = const.tile([128, 4, 128], f32, tag="outT")  # [hd, lq]
        eps_t = const.tile([128, 1], f32, tag="eps_t")
        nc.vector.memset(eps_t[:], LN_EPS)
        VB = const.tile([128, 32, 512], f32, tag="VB")
        QB = const.tile([128, 32, 64], f32, tag="QB")
        for g in range(4):
            nc.gpsimd.dma_start(QB[32 * g + 8: 32 * g + 9, :, :],
                                onesd[g])

        def hslice(t, h, cols):
            po = (h % 2) * 64
            return t[po:po + 64, h // 2, cols] if cols is not None else \
                t[po:po + 64, h // 2, :]

        # ---- vW_h -> VB rows h (independent of attention; DMAs start early)
        for h in range(8):
            vw_ps = pp.tile([128, 512], f32, tag="ps")
            nc.tensor.matmul(vw_ps[:], hslice(vhT, h, slice(0, 128)),
                             hslice(fwT_s, h, None), start=True, stop=True)
            vw_sb = stage.tile([128, 512], f32, tag="vw_sb", bufs=8)
            nc.scalar.copy(vw_sb[:], vw_ps[:])
            for g in range(4):
                nc.sync.dma_start(VB[32 * g + h: 32 * g + h + 1, :, :],
                                  vw_sb[32 * g: 32 * g + 32, :])

        # ---- qattn scores + softmax (transposes emitted later)
        qaexps = []
        for h in range(8):
            ps_q = pp.tile([64, 256], f32, tag="ps")
            nc.tensor.matmul(ps_q[:], hslice(quhT, h, None),
                             hslice(khT, h, None), start=True, stop=True)
            qnmax8 = small.tile([64, 1], f32, tag="qnmax8")
            nc.vector.tensor_reduce(qnmax8[:], ps_q[:], mybir.AxisListType.X,
                                    Alu.max, negate=True)
            nc.vector.tensor_scalar_mul(qnmax8[:], qnmax8[:], 0.125)
            qaexp = stage.tile([64, 256], f32, tag="qaexp", bufs=8)
            qrsum = small.tile([64, 1], f32, tag="qrsum")
            nc.scalar.activation(qaexp[:], ps_q[:], Act.Exp,
                                 bias=qnmax8[:], scale=0.125,
                                 accum_out=qrsum[:])
            qrrec = small.tile([64, 1], f32, tag="qrrec")
            nc.vector.reciprocal(qrrec[:], qrsum[:])
            nc.vector.tensor_scalar_mul(qaexp[:], qaexp[:], qrrec[:])
            qaexps.append(qaexp)

        # ---- attention scores + softmax
        aexps, rrecs = [], []
        for h in range(8):
            ps_s = pp.tile([128, 256], f32, tag="ps")
            nc.tensor.matmul(ps_s[:], hslice(qhT, h, None),
                             hslice(khT, h, None), start=True, stop=True)
            nmax8 = small.tile([128, 1], f32, tag="nmax8")
            nc.vector.tensor_reduce(nmax8[:], ps_s[:], mybir.AxisListType.X,
                                    Alu.max, negate=True)
            nc.vector.tensor_scalar_mul(nmax8[:], nmax8[:], 0.125)
            aexp = stage.tile([128, 256], f32, tag="aexp", bufs=8)
            rsum = small.tile([128, 1], f32, tag="rsum")
            nc.scalar.activation(aexp[:], ps_s[:], Act.Exp,
                                 bias=nmax8[:], scale=0.125, accum_out=rsum[:])
            rrec = small.tile([128, 1], f32, tag="rrec")
            nc.vector.reciprocal(rrec[:], rsum[:])
            aexps.append(aexp)
            rrecs.append(rrec)

        # ---- qattn transposes -> QB (one strided DMA per head)
        for h in range(8):
            qaT_ps = pp.tile([128, 64], f32, tag="ps")
            nc.tensor.transpose(qaT_ps[:], qaexps[h][:, 0:128],
                                id_s[:64, :64])
            qa_sb = stage.tile([128, 64], f32, tag="qa_sb", bufs=8)
            nc.vector.tensor_copy(qa_sb[:], qaT_ps[:])
            for g in range(4):
                nc.sync.dma_start(QB[32 * g + h: 32 * g + h + 1, :, :],
                                  qa_sb[32 * g: 32 * g + 32, :])

        aTs = []
        for h in range(8):
            pair = []
            for lj in range(2):
                aT_ps = pp.tile([128, 128], f32, tag="ps")
                nc.tensor.transpose(
                    aT_ps[:], aexps[h][:, lj * 128:(lj + 1) * 128], id_s[:])
                aT_sb = stage.tile([128, 128], f32, tag="aT", bufs=10)
                nc.vector.tensor_copy(aT_sb[:], aT_ps[:])
                pair.append(aT_sb)
            aTs.append(pair)
        outns = []
        for h in range(8):
            out_ps = pp.tile([128, 64], f32, tag="ps")
            for lj in range(2):
                nc.tensor.matmul(out_ps[:], aTs[h][lj][:],
                                 vh[lj][:, h * 64:(h + 1) * 64],
                                 start=(lj == 0), stop=(lj == 1))
            outn = stage.tile([128, 64], f32, tag="outn", bufs=8)
            nc.vector.tensor_scalar_mul(outn[:], out_ps[:], rrecs[h][:])
            outns.append(outn)
        for h in range(8):
            oT_ps = pp.tile([64, 128], f32, tag="ps")
            nc.tensor.transpose(oT_ps[:], outns[h][:], id_s[:])
            nc.scalar.copy(outT[(h % 2) * 64:(h % 2) * 64 + 64, h // 2, :],
                           oT_ps[:])

        # ---- outW + base
        ow_ps = pp.tile([128, 512], f32, tag="ps")
        for c in range(4):
            nc.tensor.matmul(ow_ps[:], outT[:, c, :], fwT_s[:, c, :],
                             start=(c == 0), stop=(c == 3))
        base = const.tile([128, 512], f32, tag="base")
        nc.vector.tensor_add(base[:], ow_ps[:], qpb_s[:])
        for g in range(4):
            nc.gpsimd.dma_start(VB[32 * g + 8: 32 * g + 9, :, :],
                                base[32 * g: 32 * g + 32, :])

        # ---- termB + layernorm: 16 passes, 8 lk each (4 row x 2 col tiles)
        pp_cm.__exit__(None, None, None)
        with tc.tile_pool(name="pb", bufs=8, space="PSUM") as pb:
            for u in range(16):
                banks = []
                for g in range(4):
                    bank = pb.tile([128, 512], f32, name=f"bank{u}_{g}",
                                   tag="bank")
                    banks.append(bank)
                    for c in range(2):
                        s = 2 * u + c
                        nc.tensor.matmul(
                            bank[64 * c: 64 * c + 64, :],
                            QB[32 * g: 32 * g + 9, s, :],
                            VB[32 * g: 32 * g + 9, s, :],
                            start=True, stop=True,
                            tile_position=(32 * g, 64 * c))
                st6 = small.tile([128, 6, 4], f32, tag="st6")
                st2 = small.tile([128, 2, 4], f32, tag="st2")
                for g in range(4):
                    nc.vector.bn_stats(st6[:, :, g:g + 1], banks[g][:])
                    nc.vector.bn_aggr(st2[:, :, g:g + 1], st6[:, :, g:g + 1])
                std4 = small.tile([128, 4], f32, tag="std4")
                nc.scalar.activation(std4[:], st2[:, 1, :], Act.Sqrt,
                                     bias=eps_t[:], scale=1.0)
                rstd4 = small.tile([128, 4], f32, tag="rstd4")
                nc.vector.reciprocal(rstd4[:], std4[:])
                nmr4 = small.tile([128, 4], f32, tag="nmr4")
                nc.vector.scalar_tensor_tensor(nmr4[:], st2[:, 0, :], -1.0,
                                               rstd4[:], op0=Alu.mult,
                                               op1=Alu.mult)
                big = stage.tile([128, 2048], f32, tag="big", bufs=2)
                for g in range(4):
                    nc.scalar.activation(big[:, 512 * g: 512 * g + 512],
                                         banks[g][:], Act.Identity,
                                         bias=nmr4[:, g:g + 1],
                                         scale=rstd4[:, g:g + 1])
                    if apply_affine:
                        nc.vector.tensor_mul(big[:, 512 * g: 512 * g + 512],
                                             big[:, 512 * g: 512 * g + 512],
                                             lnw_s[:])
                        nc.vector.tensor_add(big[:, 512 * g: 512 * g + 512],
                                             big[:, 512 * g: 512 * g + 512],
                                             lnb_s[:])
                nc.sync.dma_start(outd[u], big[:])

    nc.compile()
    return nc


def _get_nc(apply_affine: bool):
    if apply_affine not in _CACHE:
        _CACHE[apply_affine] = _build(apply_affine)
    return _CACHE[apply_affine]


def _decode_out(raw):
    # [16, 128, 2048] -> [nq, local lk, m]
    arr = np.asarray(raw).reshape(16, 2, 64, 4, 512)
    return arr.transpose(2, 3, 0, 1, 4).reshape(NQ, 128, 512)


def _to_dev(x):
    # [512, N] -> [128, 4, N]
    return np.ascontiguousarray(
        x.reshape(4, 128, x.shape[1]).transpose(1, 0, 2))


def make_in_maps(q, k, v, query, w_qs, w_ks, w_vs, w_query, fc_w, fc_b,
                 ln_w, ln_b, apply_affine):
    ident = np.eye(128, dtype=np.float32)
    wqT = _to_dev(w_qs.T)
    wkT = _to_dev(w_ks.T)
    wvT = _to_dev(w_vs.T)
    wuT = _to_dev(w_query.T)
    fwT = _to_dev(fc_w.T)
    in_maps = []
    for c in range(N_CORES):
        b, half = c // 2, c % 2
        L0 = 128 * half
        # swap lk halves so that local lk 0..127 is always this core's half
        perm = np.r_[L0:L0 + 128, (128 - L0):(128 - L0) + 128]
        m = {
            "kT": _to_dev(np.ascontiguousarray(k[b][perm].T)),
            "vT": _to_dev(np.ascontiguousarray(v[b][perm].T)),
            "qT": _to_dev(np.ascontiguousarray(q[b, L0:L0 + 128].T)),
            "quT": _to_dev(query[b].T),
            "wq": wqT, "wk": wkT, "wv": wvT, "wu": wuT, "fwT": fwT,
            "qpb": q[b, L0:L0 + 128] + fc_b,
            "ident": ident,
            "ones": np.ones((4, 32, 64), np.float32),
        }
        if apply_affine:
            m["lnw_r"] = np.ascontiguousarray(
                np.broadcast_to(ln_w, (128, 512)))
            m["lnb_r"] = np.ascontiguousarray(
                np.broadcast_to(ln_b, (128, 512)))
        in_maps.append(m)
    return in_maps


def kernel(q, k, v, query, w_qs, w_ks, w_vs, w_query, fc_w, fc_b, ln_w, ln_b):
    from concourse.bass_utils import run_bass_kernel_spmd

    q, k, v, query = (np.asarray(a, np.float32) for a in (q, k, v, query))
    w_qs, w_ks, w_vs, w_query, fc_w = (
        np.asarray(a, np.float32) for a in (w_qs, w_ks, w_vs, w_query, fc_w))
    fc_b, ln_w, ln_b = (np.asarray(a, np.float32)
                        for a in (fc_b, ln_w, ln_b))

    apply_affine = not (np.all(ln_w == 1.0) and np.all(ln_b == 0.0))
    nc = _get_nc(apply_affine)
    in_maps = make_in_maps(q, k, v, query, w_qs, w_ks, w_vs, w_query,
                           fc_w, fc_b, ln_w, ln_b, apply_affine)
    res = run_bass_kernel_spmd(nc, in_maps, list(range(N_CORES)))

    full = np.empty((B, NQ, LEN_K, D_MODEL), np.float32)
    for c in range(N_CORES):
        b, half = c // 2, c % 2
        full[b, :, 128 * half:128 * half + 128, :] = _decode_out(
            res.results[c]["out"])
    return full


# revision 28
# speedup vs baseline: 1.2733x; 1.0746x over previous
"""Trainium2 Bass kernel for nn_MultiHeadQuery_selfattention.

Sharding: 8 cores = 4 batches x 2 lk-halves. Core c handles batch b=c//2 and
key rows L = [128*(c%2), 128*(c%2)+128). Each core computes its output slice
out[b, :, L, :] fully independently (no collectives). A single NEFF serves
all cores: for the upper half, the host swaps the two lk-halves of k and v
(softmax/attention sums are permutation-invariant over keys), so on device
"local lk 0..127" is always the core's half.

Math per core (restructured from the reference):
  khT/vhT/qhT/quT: per-head projections kept transposed [head_dim, token]
  attn_h  = softmax(qh_h @ kh_h.T / 8)          rows = my 128 lk
  out_h   = attn_h @ vh_h                       [128, 64]
  outW    = concat_h(out_h) @ fc_w.T            [128, 512]
  base    = outW + q[b, L, :] + fc_b            [128, 512]
  vW_h    = vh_h[L] @ fc_w.T[64h:64h+64]        [128, 512]
  qattn_h = softmax(qu_h @ kh_h.T / 8)          [64, 256]
  res[nq, l, m] = base[l, m] + sum_h qattn[h, nq, l] * vW[h, l, m]
  out = layernorm(res) * ln_w + ln_b

res is produced on the PE as 128 small matmuls with K=9: stationary =
[qattn rows; ones], moving = [vW rows; base], so PSUM holds res directly.
lk pairs share one PSUM bank [128, 512] = (2 lk x 64 nq) rows; LN stats via
bn_stats/bn_aggr; normalize on ACT. QB/VB hold the K=9 operands in 4
partition groups (local lk l -> partitions 32*(l//32)+{0..8}, slot l%32).
"""

import numpy as np

N_HEAD, D_MODEL, D_K, D_V = 8, 512, 64, 64
B, LEN_K, NQ = 4, 256, 64
LN_EPS = 1e-5
N_CORES = 8

_CACHE = {}


def _build(apply_affine: bool):
    import contextlib

    import concourse.tile as tile
    from concourse import bacc, mybir

    f32 = mybir.dt.float32
    f32r = mybir.dt.float32r
    Alu = mybir.AluOpType
    Act = mybir.ActivationFunctionType

    nc = bacc.Bacc("TRN2", target_bir_lowering=False, debug=False,
                   num_devices=N_CORES)

    def din(name, shape, dt=None):
        return nc.dram_tensor(name, shape, dt or f32, kind="ExternalInput")

    # device-layout inputs ([512, N] host tensors stored as [128, 4, N])
    kT = din("kT", [128, 4, 256], f32r)
    vT = din("vT", [128, 4, 256], f32r)
    qT = din("qT", [128, 4, 128], f32r)     # only my 128 lq columns
    quT = din("quT", [128, 4, 64], f32r)
    wq = din("wq", [128, 4, 512], f32r)     # w_qs.T
    wk = din("wk", [128, 4, 512], f32r)
    wv = din("wv", [128, 4, 512], f32r)
    wu = din("wu", [128, 4, 512], f32r)
    fwT = din("fwT", [128, 4, 512], f32r)   # fc_w.T
    qpb = din("qpb", [128, 512])      # q[b, L, :] + fc_b
    ident = din("ident", [128, 128], f32r)
    onesd = din("ones", [4, 32, 64])
    if apply_affine:
        lnw_r = din("lnw_r", [128, 512])
        lnb_r = din("lnb_r", [128, 512])
    # pass u, column-block g holds rows (lkk, nq) of lk = 32g + 2u + lkk
    outd = nc.dram_tensor("out", [16, 128, 2048], f32, kind="ExternalOutput")

    with tile.TileContext(nc) as tc, contextlib.ExitStack() as ctx:
        const = ctx.enter_context(tc.tile_pool(name="const", bufs=1))
        stage = ctx.enter_context(tc.tile_pool(name="stage", bufs=4))
        small = ctx.enter_context(tc.tile_pool(name="small", bufs=8))
        pp_cm = tc.tile_pool(name="pp", bufs=8, space="PSUM")
        pp = pp_cm.__enter__()

        def load(dram, shape):
            t = const.tile(shape, dram.dtype, tag=dram.name)
            nc.sync.dma_start(t[:], dram[:])
            return t

        kT_s = load(kT, [128, 4, 256])
        vT_s = load(vT, [128, 4, 256])
        qT_s = load(qT, [128, 4, 128])
        quT_s = load(quT, [128, 4, 64])
        wq_s = load(wq, [128, 4, 512])
        wk_s = load(wk, [128, 4, 512])
        wv_s = load(wv, [128, 4, 512])
        wu_s = load(wu, [128, 4, 512])
        fwT_s = load(fwT, [128, 4, 512])
        qpb_s = load(qpb, [128, 512])
        id_s = load(ident, [128, 128])
        if apply_affine:
            lnw_s = load(lnw_r, [128, 512])
            lnb_s = load(lnb_r, [128, 512])

        # ---- projections: xhT[hd, tok] = sum_dm w.T[dm, hd] * x.T[dm, tok]
        def project(w_s, x_s, ncols, tag):
            dst = const.tile([128, 4, ncols], f32r, tag=tag)
            for mo in range(4):
                ps = pp.tile([128, ncols], f32, tag="ps")
                for ki in range(4):
                    nc.tensor.matmul(
                        ps[:], w_s[:, ki, mo * 128:(mo + 1) * 128],
                        x_s[:, ki, :], start=(ki == 0), stop=(ki == 3))
                nc.scalar.copy(dst[:, mo, :], ps[:])
            return dst

        khT = project(wk_s, kT_s, 256, "khT")
        vhT = project(wv_s, vT_s, 256, "vhT")
        qhT = project(wq_s, qT_s, 128, "qhT")
        quhT = project(wu_s, quT_s, 64, "quhT")

        # ---- vh untransposed [lk, hd] as two [128, 512] tiles
        vh = [const.tile([128, 512], f32r, name=f"vh{j}", tag=f"vh{j}")
              for j in range(2)]
        for lj in range(2):
            for hc in range(4):
                ps = pp.tile([128, 128], f32r, tag="ps")
                nc.tensor.transpose(
                    ps[:], vhT[:, hc, lj * 128:(lj + 1) * 128], id_s[:])
                nc.scalar.copy(vh[lj][:, hc * 128:(hc + 1) * 128], ps[:])

        # ---- per-head attention
        outT = const.tile([128, 4, 128], f32r, tag="outT")  # [hd, lq]
        eps_t = const.tile([128, 1], f32, tag="eps_t")
        nc.vector.memset(eps_t[:], LN_EPS)
        VB = const.tile([128, 32, 512], f32, tag="VB")
        QB = const.tile([128, 32, 64], f32, tag="QB")
        for g in range(4):
            nc.gpsimd.dma_start(QB[32 * g + 8: 32 * g + 9, :, :],
                                onesd[g])

        def hslice(t, h, cols):
            po = (h % 2) * 64
            return t[po:po + 64, h // 2, cols] if cols is not None else \
                t[po:po + 64, h // 2, :]

        # ---- vW_h -> VB rows h (independent of attention; DMAs start early)
        for h in range(8):
            vw_ps = pp.tile([128, 512], f32, tag="ps")
            nc.tensor.matmul(vw_ps[:], hslice(vhT, h, slice(0, 128)),
                             hslice(fwT_s, h, None), start=True, stop=True)
            vw_sb = stage.tile([128, 512], f32, tag="vw_sb", bufs=8)
            nc.scalar.copy(vw_sb[:], vw_ps[:])
            for g in range(4):
                nc.sync.dma_start(VB[32 * g + h: 32 * g + h + 1, :, :],
                                  vw_sb[32 * g: 32 * g + 32, :])

        # ---- qattn scores + softmax (transposes emitted later)
        qaexps = []
        for h in range(8):
            ps_q = pp.tile([64, 256], f32, tag="ps")
            nc.tensor.matmul(ps_q[:], hslice(quhT, h, None),
                             hslice(khT, h, None), start=True, stop=True)
            qnmax8 = small.tile([64, 1], f32, tag="qnmax8")
            nc.vector.tensor_reduce(qnmax8[:], ps_q[:], mybir.AxisListType.X,
                                    Alu.max, negate=True)
            nc.vector.tensor_scalar_mul(qnmax8[:], qnmax8[:], 0.125)
            qaexp = stage.tile([64, 256], f32, tag="qaexp", bufs=8)
            qrsum = small.tile([64, 1], f32, tag="qrsum")
            nc.scalar.activation(qaexp[:], ps_q[:], Act.Exp,
                                 bias=qnmax8[:], scale=0.125,
                                 accum_out=qrsum[:])
            qrrec = small.tile([64, 1], f32, tag="qrrec")
            nc.vector.reciprocal(qrrec[:], qrsum[:])
            nc.vector.tensor_scalar_mul(qaexp[:], qaexp[:], qrrec[:])
            qaexps.append(qaexp)

        # ---- attention scores + softmax
        aexps, rrecs = [], []
        for h in range(8):
            ps_s = pp.tile([128, 256], f32, tag="ps")
            nc.tensor.matmul(ps_s[:], hslice(qhT, h, None),
                             hslice(khT, h, None), start=True, stop=True)
            nmax8 = small.tile([128, 1], f32, tag="nmax8")
            nc.vector.tensor_reduce(nmax8[:], ps_s[:], mybir.AxisListType.X,
                                    Alu.max, negate=True)
            nc.vector.tensor_scalar_mul(nmax8[:], nmax8[:], 0.125)
            aexp = stage.tile([128, 256], f32r, tag="aexp", bufs=8)
            rsum = small.tile([128, 1], f32, tag="rsum")
            nc.scalar.activation(aexp[:], ps_s[:], Act.Exp,
                                 bias=nmax8[:], scale=0.125, accum_out=rsum[:])
            rrec = small.tile([128, 1], f32, tag="rrec")
            nc.vector.reciprocal(rrec[:], rsum[:])
            aexps.append(aexp)
            rrecs.append(rrec)

        # ---- qattn transposes -> QB (one strided DMA per head)
        for h in range(8):
            qaT_ps = pp.tile([128, 64], f32, tag="ps")
            nc.tensor.transpose(qaT_ps[:], qaexps[h][:, 0:128],
                                id_s[:64, :64].bitcast(f32))
            qa_sb = stage.tile([128, 64], f32, tag="qa_sb", bufs=8)
            nc.vector.tensor_copy(qa_sb[:], qaT_ps[:])
            for g in range(4):
                nc.sync.dma_start(QB[32 * g + h: 32 * g + h + 1, :, :],
                                  qa_sb[32 * g: 32 * g + 32, :])

        aTs = []
        for h in range(8):
            pair = []
            for lj in range(2):
                aT_ps = pp.tile([128, 128], f32r, tag="ps")
                nc.tensor.transpose(
                    aT_ps[:], aexps[h][:, lj * 128:(lj + 1) * 128], id_s[:])
                aT_sb = stage.tile([128, 128], f32r, tag="aT", bufs=10)
                nc.vector.tensor_copy(aT_sb[:], aT_ps[:])
                pair.append(aT_sb)
            aTs.append(pair)
        outns = []
        for h in range(8):
            out_ps = pp.tile([128, 64], f32, tag="ps")
            for lj in range(2):
                nc.tensor.matmul(out_ps[:], aTs[h][lj][:],
                                 vh[lj][:, h * 64:(h + 1) * 64],
                                 start=(lj == 0), stop=(lj == 1))
            outn = stage.tile([128, 64], f32r, tag="outn", bufs=8)
            nc.vector.tensor_scalar_mul(outn[:], out_ps[:], rrecs[h][:])
            outns.append(outn)
        for h in range(8):
            oT_ps = pp.tile([64, 128], f32r, tag="ps")
            nc.tensor.transpose(oT_ps[:], outns[h][:], id_s[:])
            nc.scalar.copy(outT[(h % 2) * 64:(h % 2) * 64 + 64, h // 2, :],
                           oT_ps[:])

        # ---- outW + base
        ow_ps = pp.tile([128, 512], f32, tag="ps")
        for c in range(4):
            nc.tensor.matmul(ow_ps[:], outT[:, c, :], fwT_s[:, c, :],
                             start=(c == 0), stop=(c == 3))
        base = const.tile([128, 512], f32, tag="base")
        nc.vector.tensor_add(base[:], ow_ps[:], qpb_s[:])
        for g in range(4):
            nc.gpsimd.dma_start(VB[32 * g + 8: 32 * g + 9, :, :],
                                base[32 * g: 32 * g + 32, :])

        # ---- termB + layernorm: 16 passes, 8 lk each (4 row x 2 col tiles)
        pp_cm.__exit__(None, None, None)
        with tc.tile_pool(name="pb", bufs=8, space="PSUM") as pb:
            for u in range(16):
                banks = []
                for g in range(4):
                    bank = pb.tile([128, 512], f32, name=f"bank{u}_{g}",
                                   tag="bank")
                    banks.append(bank)
                    for c in range(2):
                        s = 2 * u + c
                        nc.tensor.matmul(
                            bank[64 * c: 64 * c + 64, :],
                            QB[32 * g: 32 * g + 9, s, :],
                            VB[32 * g: 32 * g + 9, s, :],
                            start=True, stop=True,
                            tile_position=(32 * g, 64 * c))
                st6 = small.tile([128, 6, 4], f32, tag="st6")
                st2 = small.tile([128, 2, 4], f32, tag="st2")
                for g in range(4):
                    nc.vector.bn_stats(st6[:, :, g:g + 1], banks[g][:])
                    nc.vector.bn_aggr(st2[:, :, g:g + 1], st6[:, :, g:g + 1])
                std4 = small.tile([128, 4], f32, tag="std4")
                nc.scalar.activation(std4[:], st2[:, 1, :], Act.Sqrt,
                                     bias=eps_t[:], scale=1.0)
                rstd4 = small.tile([128, 4], f32, tag="rstd4")
                nc.vector.reciprocal(rstd4[:], std4[:])
                nmr4 = small.tile([128, 4], f32, tag="nmr4")
                nc.vector.scalar_tensor_tensor(nmr4[:], st2[:, 0, :], -1.0,
                                               rstd4[:], op0=Alu.mult,
                                               op1=Alu.mult)
                big = stage.tile([128, 2048], f32, tag="big", bufs=2)
                for g in range(4):
                    nc.scalar.activation(big[:, 512 * g: 512 * g + 512],
                                         banks[g][:], Act.Identity,
                                         bias=nmr4[:, g:g + 1],
                                         scale=rstd4[:, g:g + 1])
                    if apply_affine:
                        nc.vector.tensor_mul(big[:, 512 * g: 512 * g + 512],
                                             big[:, 512 * g: 512 * g + 512],
                                             lnw_s[:])
                        nc.vector.tensor_add(big[:, 512 * g: 512 * g + 512],
                                             big[:, 512 * g: 512 * g + 512],
                                             lnb_s[:])
                nc.sync.dma_start(outd[u], big[:])

    nc.compile()
    return nc


def _get_nc(apply_affine: bool):
    if apply_affine not in _CACHE:
        _CACHE[apply_affine] = _build(apply_affine)
    return _CACHE[apply_affine]


def _decode_out(raw):
    # [16, 128, 2048] -> [nq, local lk, m]
    arr = np.asarray(raw).reshape(16, 2, 64, 4, 512)
    return arr.transpose(2, 3, 0, 1, 4).reshape(NQ, 128, 512)


def _to_dev(x):
    # [512, N] -> [128, 4, N]
    return np.ascontiguousarray(
        x.reshape(4, 128, x.shape[1]).transpose(1, 0, 2))


def make_in_maps(q, k, v, query, w_qs, w_ks, w_vs, w_query, fc_w, fc_b,
                 ln_w, ln_b, apply_affine):
    ident = np.eye(128, dtype=np.float32)
    wqT = _to_dev(w_qs.T)
    wkT = _to_dev(w_ks.T)
    wvT = _to_dev(w_vs.T)
    wuT = _to_dev(w_query.T)
    fwT = _to_dev(fc_w.T)
    in_maps = []
    for c in range(N_CORES):
        b, half = c // 2, c % 2
        L0 = 128 * half
        # swap lk halves so that local lk 0..127 is always this core's half
        perm = np.r_[L0:L0 + 128, (128 - L0):(128 - L0) + 128]
        m = {
            "kT": _to_dev(np.ascontiguousarray(k[b][perm].T)),
            "vT": _to_dev(np.ascontiguousarray(v[b][perm].T)),
            "qT": _to_dev(np.ascontiguousarray(q[b, L0:L0 + 128].T)),
            "quT": _to_dev(query[b].T),
            "wq": wqT, "wk": wkT, "wv": wvT, "wu": wuT, "fwT": fwT,
            "qpb": q[b, L0:L0 + 128] + fc_b,
            "ident": ident,
            "ones": np.ones((4, 32, 64), np.float32),
        }
        if apply_affine:
            m["lnw_r"] = np.ascontiguousarray(
                np.broadcast_to(ln_w, (128, 512)))
            m["lnb_r"] = np.ascontiguousarray(
                np.broadcast_to(ln_b, (128, 512)))
        in_maps.append(m)
    return in_maps


def kernel(q, k, v, query, w_qs, w_ks, w_vs, w_query, fc_w, fc_b, ln_w, ln_b):
    from concourse.bass_utils import run_bass_kernel_spmd

    q, k, v, query = (np.asarray(a, np.float32) for a in (q, k, v, query))
    w_qs, w_ks, w_vs, w_query, fc_w = (
        np.asarray(a, np.float32) for a in (w_qs, w_ks, w_vs, w_query, fc_w))
    fc_b, ln_w, ln_b = (np.asarray(a, np.float32)
                        for a in (fc_b, ln_w, ln_b))

    apply_affine = not (np.all(ln_w == 1.0) and np.all(ln_b == 0.0))
    nc = _get_nc(apply_affine)
    in_maps = make_in_maps(q, k, v, query, w_qs, w_ks, w_vs, w_query,
                           fc_w, fc_b, ln_w, ln_b, apply_affine)
    res = run_bass_kernel_spmd(nc, in_maps, list(range(N_CORES)))

    full = np.empty((B, NQ, LEN_K, D_MODEL), np.float32)
    for c in range(N_CORES):
        b, half = c // 2, c % 2
        full[b, :, 128 * half:128 * half + 128, :] = _decode_out(
            res.results[c]["out"])
    return full


# revision 33
# speedup vs baseline: 1.3794x; 1.0833x over previous
"""Trainium2 Bass kernel for nn_MultiHeadQuery_selfattention.

Sharding: 8 cores = 4 batches x 2 lk-halves. Core c handles batch b=c//2 and
key rows L = [128*(c%2), 128*(c%2)+128). Each core computes its output slice
out[b, :, L, :] fully independently (no collectives). A single NEFF serves
all cores: for the upper half, the host swaps the two lk-halves of k and v
(softmax/attention sums are permutation-invariant over keys), so on device
"local lk 0..127" is always the core's half.

Math per core (restructured from the reference):
  khT/vhT/qhT/quT: per-head projections kept transposed [head_dim, token]
  attn_h  = softmax(qh_h @ kh_h.T / 8)          rows = my 128 lk
  out_h   = attn_h @ vh_h                       [128, 64]
  outW    = concat_h(out_h) @ fc_w.T            [128, 512]
  base    = outW + q[b, L, :] + fc_b            [128, 512]
  vW_h    = vh_h[L] @ fc_w.T[64h:64h+64]        [128, 512]
  qattn_h = softmax(qu_h @ kh_h.T / 8)          [64, 256]
  res[nq, l, m] = base[l, m] + sum_h qattn[h, nq, l] * vW[h, l, m]
  out = layernorm(res) * ln_w + ln_b

res is produced on the PE as 128 small matmuls with K=9: stationary =
[qattn rows; ones], moving = [vW rows; base], so PSUM holds res directly.
lk pairs share one PSUM bank [128, 512] = (2 lk x 64 nq) rows; LN stats via
bn_stats/bn_aggr; normalize on ACT. QB/VB hold the K=9 operands in 4
partition groups (local lk l -> partitions 32*(l//32)+{0..8}, slot l%32).
"""

import numpy as np

N_HEAD, D_MODEL, D_K, D_V = 8, 512, 64, 64
B, LEN_K, NQ = 4, 256, 64
LN_EPS = 1e-5
N_CORES = 8

_CACHE = {}


def _build(apply_affine: bool):
    import contextlib

    import concourse.tile as tile
    from concourse import bacc, mybir

    f32 = mybir.dt.float32
    f32r = mybir.dt.float32r
    Alu = mybir.AluOpType
    Act = mybir.ActivationFunctionType

    nc = bacc.Bacc("TRN2", target_bir_lowering=False, debug=False,
                   num_devices=N_CORES)

    def din(name, shape, dt=None):
        return nc.dram_tensor(name, shape, dt or f32, kind="ExternalInput")

    # device-layout inputs ([512, N] host tensors stored as [128, 4, N])
    kT = din("kT", [128, 4, 256], f32r)
    vT = din("vT", [128, 4, 256], f32r)
    qT = din("qT", [128, 4, 128], f32r)     # only my 128 lq columns
    quT = din("quT", [128, 4, 64], f32r)
    wq = din("wq", [128, 4, 512], f32r)     # w_qs.T
    wk = din("wk", [128, 4, 512], f32r)
    wv = din("wv", [128, 4, 512], f32r)
    wu = din("wu", [128, 4, 512], f32r)
    fwT = din("fwT", [128, 4, 512], f32r)   # fc_w.T
    qpb = din("qpb", [128, 512])      # q[b, L, :] + fc_b
    ident = din("ident", [128, 128], f32r)
    if apply_affine:
        lnw_r = din("lnw_r", [128, 512])
        lnb_r = din("lnb_r", [128, 512])
    # pass u, column-block g holds rows (lkk, nq) of lk = 32g + 2u + lkk
    outd = nc.dram_tensor("out", [16, 128, 2048], f32, kind="ExternalOutput")

    with tile.TileContext(nc) as tc, contextlib.ExitStack() as ctx:
        const = ctx.enter_context(tc.tile_pool(name="const", bufs=1))
        stage = ctx.enter_context(tc.tile_pool(name="stage", bufs=4))
        small = ctx.enter_context(tc.tile_pool(name="small", bufs=8))
        pp_cm = tc.tile_pool(name="pp", bufs=8, space="PSUM")
        pp = pp_cm.__enter__()

        def load(dram, shape):
            t = const.tile(shape, dram.dtype, tag=dram.name)
            nc.sync.dma_start(t[:], dram[:])
            return t

        kT_s = load(kT, [128, 4, 256])
        vT_s = load(vT, [128, 4, 256])
        qT_s = load(qT, [128, 4, 128])
        quT_s = load(quT, [128, 4, 64])
        wq_s = load(wq, [128, 4, 512])
        wk_s = load(wk, [128, 4, 512])
        wv_s = load(wv, [128, 4, 512])
        wu_s = load(wu, [128, 4, 512])
        fwT_s = load(fwT, [128, 4, 512])
        qpb_s = load(qpb, [128, 512])
        id_s = load(ident, [128, 128])
        if apply_affine:
            lnw_s = load(lnw_r, [128, 512])
            lnb_s = load(lnb_r, [128, 512])

        # ---- projections: xhT[hd, tok] = sum_dm w.T[dm, hd] * x.T[dm, tok]
        def project(w_s, x_s, ncols, tag):
            dst = const.tile([128, 4, ncols], f32r, tag=tag)
            for mo in range(4):
                ps = pp.tile([128, ncols], f32, tag="ps")
                for ki in range(4):
                    nc.tensor.matmul(
                        ps[:], w_s[:, ki, mo * 128:(mo + 1) * 128],
                        x_s[:, ki, :], start=(ki == 0), stop=(ki == 3))
                nc.scalar.copy(dst[:, mo, :], ps[:])
            return dst

        khT = project(wk_s, kT_s, 256, "khT")
        vhT = project(wv_s, vT_s, 256, "vhT")
        qhT = project(wq_s, qT_s, 128, "qhT")
        quhT = project(wu_s, quT_s, 64, "quhT")

        # ---- vh untransposed [lk, hd] as two [128, 512] tiles
        vh = [const.tile([128, 512], f32r, name=f"vh{j}", tag=f"vh{j}")
              for j in range(2)]
        for lj in range(2):
            for hc in range(4):
                ps = pp.tile([128, 128], f32r, tag="ps")
                nc.tensor.transpose(
                    ps[:], vhT[:, hc, lj * 128:(lj + 1) * 128], id_s[:])
                nc.scalar.copy(vh[lj][:, hc * 128:(hc + 1) * 128], ps[:])

        # ---- per-head attention
        outT = const.tile([128, 4, 128], f32r, tag="outT")  # [hd, lq]
        eps_t = const.tile([128, 1], f32, tag="eps_t")
        nc.vector.memset(eps_t[:], LN_EPS)
        VB = const.tile([128, 32, 512], f32, tag="VB")
        QBT = const.tile([128, 32, 64], f32, tag="QBT")


        def hslice(t, h, cols):
            po = (h % 2) * 64
            return t[po:po + 64, h // 2, cols] if cols is not None else \
                t[po:po + 64, h // 2, :]

        # ---- vW_h -> VB rows h (independent of attention; DMAs start early)
        for h in range(8):
            vw_ps = pp.tile([128, 512], f32, tag="ps")
            nc.tensor.matmul(vw_ps[:], hslice(vhT, h, slice(0, 128)),
                             hslice(fwT_s, h, None), start=True, stop=True)
            vw_sb = stage.tile([128, 512], f32, tag="vw_sb", bufs=6)
            nc.scalar.copy(vw_sb[:], vw_ps[:])
            for g in range(4):
                eng = nc.sync if g % 2 == 0 else nc.gpsimd
                eng.dma_start(VB[32 * g + h: 32 * g + h + 1, :, :],
                              vw_sb[32 * g: 32 * g + 32, :])

        # ---- qattn scores + softmax (transposes emitted later)
        qaexps = []
        for h in range(8):
            ps_q = pp.tile([64, 256], f32, tag="ps")
            nc.tensor.matmul(ps_q[:], hslice(quhT, h, None),
                             hslice(khT, h, None), start=True, stop=True)
            qnmax8 = small.tile([64, 1], f32, tag="qnmax8")
            nc.vector.tensor_reduce(qnmax8[:], ps_q[:], mybir.AxisListType.X,
                                    Alu.max, negate=True)
            nc.vector.tensor_scalar_mul(qnmax8[:], qnmax8[:], 0.125)
            qaexp = stage.tile([64, 256], f32, tag="qaexp", bufs=8)
            qrsum = small.tile([64, 1], f32, tag="qrsum")
            nc.scalar.activation(qaexp[:], ps_q[:], Act.Exp,
                                 bias=qnmax8[:], scale=0.125,
                                 accum_out=qrsum[:])
            qrrec = small.tile([64, 1], f32, tag="qrrec")
            nc.vector.reciprocal(qrrec[:], qrsum[:])
            nc.vector.tensor_scalar_mul(qaexp[:], qaexp[:], qrrec[:])
            qaexps.append(qaexp)

        # ---- attention scores + softmax
        aexps, rrecs = [], []
        for h in range(8):
            ps_s = pp.tile([128, 256], f32, tag="ps")
            nc.tensor.matmul(ps_s[:], hslice(qhT, h, None),
                             hslice(khT, h, None), start=True, stop=True)
            nmax8 = small.tile([128, 1], f32, tag="nmax8")
            nc.vector.tensor_reduce(nmax8[:], ps_s[:], mybir.AxisListType.X,
                                    Alu.max, negate=True)
            nc.vector.tensor_scalar_mul(nmax8[:], nmax8[:], 0.125)
            aexp = stage.tile([128, 256], f32r, tag="aexp", bufs=8)
            rsum = small.tile([128, 1], f32, tag="rsum")
            nc.scalar.activation(aexp[:], ps_s[:], Act.Exp,
                                 bias=nmax8[:], scale=0.125, accum_out=rsum[:])
            rrec = small.tile([128, 1], f32, tag="rrec")
            nc.vector.reciprocal(rrec[:], rsum[:])
            aexps.append(aexp)
            rrecs.append(rrec)

        # ---- qattn -> QBT on-chip: interleave columns, then PE transpose.
        # qcat[:, sh, 32g + h] = qattn[h, nq, 32g + 16*half + sh] (f32r)
        for half in range(2):
            qcat = stage.tile([64, 16, 128], f32r, name=f"qcat{half}",
                              tag="qcat", bufs=1)
            nc.vector.memset(qcat[:].bitcast(f32), 1.0)
            for h in range(8):
                for g in range(4):
                    nc.vector.tensor_copy(
                        qcat[:, :, 32 * g + h],
                        qaexps[h][:, 32 * g + 16 * half:
                                   32 * g + 16 * half + 16])
            for sh in range(16):
                s = 16 * half + sh
                qt_ps = pp.tile([128, 64], f32r, tag="ps")
                nc.tensor.transpose(qt_ps[:], qcat[:, sh, :], id_s[:64, :64])
                nc.vector.tensor_copy(QBT[:, s, :], qt_ps[:])

        aTs = []
        for h in range(8):
            pair = []
            for lj in range(2):
                aT_ps = pp.tile([128, 128], f32r, tag="ps")
                nc.tensor.transpose(
                    aT_ps[:], aexps[h][:, lj * 128:(lj + 1) * 128], id_s[:])
                aT_sb = stage.tile([128, 128], f32r, tag="aT", bufs=10)
                nc.vector.tensor_copy(aT_sb[:], aT_ps[:])
                pair.append(aT_sb)
            aTs.append(pair)
        outns = []
        for h in range(8):
            out_ps = pp.tile([128, 64], f32, tag="ps")
            for lj in range(2):
                nc.tensor.matmul(out_ps[:], aTs[h][lj][:],
                                 vh[lj][:, h * 64:(h + 1) * 64],
                                 start=(lj == 0), stop=(lj == 1))
            outn = stage.tile([128, 64], f32r, tag="outn", bufs=8)
            nc.vector.tensor_scalar_mul(outn[:], out_ps[:], rrecs[h][:])
            outns.append(outn)
        for h in range(8):
            oT_ps = pp.tile([64, 128], f32r, tag="ps")
            nc.tensor.transpose(oT_ps[:], outns[h][:], id_s[:])
            nc.scalar.copy(outT[(h % 2) * 64:(h % 2) * 64 + 64, h // 2, :],
                           oT_ps[:])

        # ---- outW + base
        ow_ps = pp.tile([128, 512], f32, tag="ps")
        for c in range(4):
            nc.tensor.matmul(ow_ps[:], outT[:, c, :], fwT_s[:, c, :],
                             start=(c == 0), stop=(c == 3))
        base = const.tile([128, 512], f32, tag="base")
        nc.vector.tensor_add(base[:], ow_ps[:], qpb_s[:])
        for g in range(4):
            nc.sync.dma_start(VB[32 * g + 8: 32 * g + 9, :, :],
                              base[32 * g: 32 * g + 32, :])

        # ---- termB + layernorm: 16 passes, 8 lk each (4 row x 2 col tiles)
        pp_cm.__exit__(None, None, None)
        with tc.tile_pool(name="pb", bufs=8, space="PSUM") as pb:
            for u in range(16):
                banks = []
                for g in range(4):
                    bank = pb.tile([128, 512], f32, name=f"bank{u}_{g}",
                                   tag="bank")
                    banks.append(bank)
                    for c in range(2):
                        s = 2 * u + c
                        nc.tensor.matmul(
                            bank[64 * c: 64 * c + 64, :],
                            QBT[32 * g: 32 * g + 9, s, :],
                            VB[32 * g: 32 * g + 9, s, :],
                            start=True, stop=True,
                            tile_position=(32 * g, 64 * c))
                st6 = small.tile([128, 6, 4], f32, tag="st6")
                st2 = small.tile([128, 2, 4], f32, tag="st2")
                for g in range(4):
                    nc.vector.bn_stats(st6[:, :, g:g + 1], banks[g][:])
                    nc.vector.bn_aggr(st2[:, :, g:g + 1], st6[:, :, g:g + 1])
                std4 = small.tile([128, 4], f32, tag="std4")
                nc.scalar.activation(std4[:], st2[:, 1, :], Act.Sqrt,
                                     bias=eps_t[:], scale=1.0)
                rstd4 = small.tile([128, 4], f32, tag="rstd4")
                nc.vector.reciprocal(rstd4[:], std4[:])
                nmr4 = small.tile([128, 4], f32, tag="nmr4")
                nc.vector.scalar_tensor_tensor(nmr4[:], st2[:, 0, :], -1.0,
                                               rstd4[:], op0=Alu.mult,
                                               op1=Alu.mult)
                big = stage.tile([128, 2048], f32, tag="big", bufs=2)
                for g in range(4):
                    nc.scalar.activation(big[:, 512 * g: 512 * g + 512],
                                         banks[g][:], Act.Identity,
                                         bias=nmr4[:, g:g + 1],
                                         scale=rstd4[:, g:g + 1])
                    if apply_affine:
                        nc.vector.tensor_mul(big[:, 512 * g: 512 * g + 512],
                                             big[:, 512 * g: 512 * g + 512],
                                             lnw_s[:])
                        nc.vector.tensor_add(big[:, 512 * g: 512 * g + 512],
                                             big[:, 512 * g: 512 * g + 512],
                                             lnb_s[:])
                nc.sync.dma_start(outd[u], big[:])

    nc.compile()
    return nc


def _get_nc(apply_affine: bool):
    if apply_affine not in _CACHE:
        _CACHE[apply_affine] = _build(apply_affine)
    return _CACHE[apply_affine]


def _decode_out(raw):
    # [u, c, nq, g, m] -> [nq, lk = 32g + 2u + c, m]
    arr = np.asarray(raw).reshape(16, 2, 64, 4, 512)
    return arr.transpose(2, 3, 0, 1, 4).reshape(NQ, 128, 512)


def _to_dev(x):
    # [512, N] -> [128, 4, N]
    return np.ascontiguousarray(
        x.reshape(4, 128, x.shape[1]).transpose(1, 0, 2))


def make_in_maps(q, k, v, query, w_qs, w_ks, w_vs, w_query, fc_w, fc_b,
                 ln_w, ln_b, apply_affine):
    ident = np.eye(128, dtype=np.float32)
    wqT = _to_dev(w_qs.T)
    wkT = _to_dev(w_ks.T)
    wvT = _to_dev(w_vs.T)
    wuT = _to_dev(w_query.T)
    fwT = _to_dev(fc_w.T)
    in_maps = []
    for c in range(N_CORES):
        b, half = c // 2, c % 2
        L0 = 128 * half
        # swap lk halves so that local lk 0..127 is always this core's half
        perm = np.r_[L0:L0 + 128, (128 - L0):(128 - L0) + 128]
        m = {
            "kT": _to_dev(np.ascontiguousarray(k[b][perm].T)),
            "vT": _to_dev(np.ascontiguousarray(v[b][perm].T)),
            "qT": _to_dev(np.ascontiguousarray(q[b, L0:L0 + 128].T)),
            "quT": _to_dev(query[b].T),
            "wq": wqT, "wk": wkT, "wv": wvT, "wu": wuT, "fwT": fwT,
            "qpb": q[b, L0:L0 + 128] + fc_b,
            "ident": ident,
        }
        if apply_affine:
            m["lnw_r"] = np.ascontiguousarray(
                np.broadcast_to(ln_w, (128, 512)))
            m["lnb_r"] = np.ascontiguousarray(
                np.broadcast_to(ln_b, (128, 512)))
        in_maps.append(m)
    return in_maps


def kernel(q, k, v, query, w_qs, w_ks, w_vs, w_query, fc_w, fc_b, ln_w, ln_b):
    from concourse.bass_utils import run_bass_kernel_spmd

    q, k, v, query = (np.asarray(a, np.float32) for a in (q, k, v, query))
    w_qs, w_ks, w_vs, w_query, fc_w = (
        np.asarray(a, np.float32) for a in (w_qs, w_ks, w_vs, w_query, fc_w))
    fc_b, ln_w, ln_b = (np.asarray(a, np.float32)
                        for a in (fc_b, ln_w, ln_b))

    apply_affine = not (np.all(ln_w == 1.0) and np.all(ln_b == 0.0))
    nc = _get_nc(apply_affine)
    in_maps = make_in_maps(q, k, v, query, w_qs, w_ks, w_vs, w_query,
                           fc_w, fc_b, ln_w, ln_b, apply_affine)
    res = run_bass_kernel_spmd(nc, in_maps, list(range(N_CORES)))

    full = np.empty((B, NQ, LEN_K, D_MODEL), np.float32)
    for c in range(N_CORES):
        b, half = c // 2, c % 2
        full[b, :, 128 * half:128 * half + 128, :] = _decode_out(
            res.results[c]["out"])
    return full


# revision 34
# speedup vs baseline: 1.4017x; 1.0161x over previous
"""Trainium2 Bass kernel for nn_MultiHeadQuery_selfattention.

Sharding: 8 cores = 4 batches x 2 lk-halves. Core c handles batch b=c//2 and
key rows L = [128*(c%2), 128*(c%2)+128). Each core computes its output slice
out[b, :, L, :] fully independently (no collectives). A single NEFF serves
all cores: for the upper half, the host swaps the two lk-halves of k and v
(softmax/attention sums are permutation-invariant over keys), so on device
"local lk 0..127" is always the core's half.

Math per core (restructured from the reference):
  khT/vhT/qhT/quT: per-head projections kept transposed [head_dim, token]
  attn_h  = softmax(qh_h @ kh_h.T / 8)          rows = my 128 lk
  out_h   = attn_h @ vh_h                       [128, 64]
  outW    = concat_h(out_h) @ fc_w.T            [128, 512]
  base    = outW + q[b, L, :] + fc_b            [128, 512]
  vW_h    = vh_h[L] @ fc_w.T[64h:64h+64]        [128, 512]
  qattn_h = softmax(qu_h @ kh_h.T / 8)          [64, 256]
  res[nq, l, m] = base[l, m] + sum_h qattn[h, nq, l] * vW[h, l, m]
  out = layernorm(res) * ln_w + ln_b

res is produced on the PE as 128 small matmuls with K=9: stationary =
[qattn rows; ones], moving = [vW rows; base], so PSUM holds res directly.
lk pairs share one PSUM bank [128, 512] = (2 lk x 64 nq) rows; LN stats via
bn_stats/bn_aggr; normalize on ACT. QB/VB hold the K=9 operands in 4
partition groups (local lk l -> partitions 32*(l//32)+{0..8}, slot l%32).
"""

import numpy as np

N_HEAD, D_MODEL, D_K, D_V = 8, 512, 64, 64
B, LEN_K, NQ = 4, 256, 64
LN_EPS = 1e-5
N_CORES = 8

_CACHE = {}


def _build(apply_affine: bool):
    import contextlib

    import concourse.tile as tile
    from concourse import bacc, mybir

    f32 = mybir.dt.float32
    f32r = mybir.dt.float32r
    Alu = mybir.AluOpType
    Act = mybir.ActivationFunctionType

    nc = bacc.Bacc("TRN2", target_bir_lowering=False, debug=False,
                   num_devices=N_CORES)

    def din(name, shape, dt=None):
        return nc.dram_tensor(name, shape, dt or f32, kind="ExternalInput")

    # device-layout inputs ([512, N] host tensors stored as [128, 4, N])
    kT = din("kT", [128, 4, 256], f32r)
    vT = din("vT", [128, 4, 256], f32r)
    qT = din("qT", [128, 4, 128], f32r)     # only my 128 lq columns
    quT = din("quT", [128, 4, 64], f32r)
    wq = din("wq", [128, 4, 512], f32r)     # w_qs.T
    wk = din("wk", [128, 4, 512], f32r)
    wv = din("wv", [128, 4, 512], f32r)
    wu = din("wu", [128, 4, 512], f32r)
    fwT = din("fwT", [128, 4, 512], f32r)   # fc_w.T
    qpb = din("qpb", [128, 512])      # q[b, L, :] + fc_b
    ident = din("ident", [128, 128], f32r)
    if apply_affine:
        lnw_r = din("lnw_r", [128, 512])
        lnb_r = din("lnb_r", [128, 512])
    # pass u, column-block g holds rows (lkk, nq) of lk = 32g + 2u + lkk
    outd = nc.dram_tensor("out", [16, 128, 2048], f32, kind="ExternalOutput")

    with tile.TileContext(nc) as tc, contextlib.ExitStack() as ctx:
        const = ctx.enter_context(tc.tile_pool(name="const", bufs=1))
        stage = ctx.enter_context(tc.tile_pool(name="stage", bufs=4))
        small = ctx.enter_context(tc.tile_pool(name="small", bufs=8))
        pp_cm = tc.tile_pool(name="pp", bufs=8, space="PSUM")
        pp = pp_cm.__enter__()

        def load(dram, shape):
            t = const.tile(shape, dram.dtype, tag=dram.name)
            nc.sync.dma_start(t[:], dram[:])
            return t

        kT_s = load(kT, [128, 4, 256])
        vT_s = load(vT, [128, 4, 256])
        qT_s = load(qT, [128, 4, 128])
        quT_s = load(quT, [128, 4, 64])
        wq_s = load(wq, [128, 4, 512])
        wk_s = load(wk, [128, 4, 512])
        wv_s = load(wv, [128, 4, 512])
        wu_s = load(wu, [128, 4, 512])
        fwT_s = load(fwT, [128, 4, 512])
        qpb_s = load(qpb, [128, 512])
        id_s = load(ident, [128, 128])
        if apply_affine:
            lnw_s = load(lnw_r, [128, 512])
            lnb_s = load(lnb_r, [128, 512])

        # ---- projections: xhT[hd, tok] = sum_dm w.T[dm, hd] * x.T[dm, tok]
        def project(w_s, x_s, ncols, tag):
            dst = const.tile([128, 4, ncols], f32r, tag=tag)
            for mo in range(4):
                ps = pp.tile([128, ncols], f32, tag="ps")
                for ki in range(4):
                    nc.tensor.matmul(
                        ps[:], w_s[:, ki, mo * 128:(mo + 1) * 128],
                        x_s[:, ki, :], start=(ki == 0), stop=(ki == 3))
                nc.scalar.copy(dst[:, mo, :], ps[:])
            return dst

        khT = project(wk_s, kT_s, 256, "khT")
        vhT = project(wv_s, vT_s, 256, "vhT")
        qhT = project(wq_s, qT_s, 128, "qhT")
        quhT = project(wu_s, quT_s, 64, "quhT")

        # ---- vh untransposed [lk, hd] as two [128, 512] tiles
        vh = [const.tile([128, 512], f32r, name=f"vh{j}", tag=f"vh{j}")
              for j in range(2)]
        for lj in range(2):
            for hc in range(4):
                ps = pp.tile([128, 128], f32r, tag="ps")
                nc.tensor.transpose(
                    ps[:], vhT[:, hc, lj * 128:(lj + 1) * 128], id_s[:])
                nc.scalar.copy(vh[lj][:, hc * 128:(hc + 1) * 128], ps[:])

        # ---- per-head attention
        outT = const.tile([128, 4, 128], f32r, tag="outT")  # [hd, lq]
        eps_t = const.tile([128, 1], f32, tag="eps_t")
        nc.vector.memset(eps_t[:], LN_EPS)
        VB = const.tile([128, 32, 512], f32, tag="VB")
        QBT = const.tile([128, 32, 64], f32, tag="QBT")


        def hslice(t, h, cols):
            po = (h % 2) * 64
            return t[po:po + 64, h // 2, cols] if cols is not None else \
                t[po:po + 64, h // 2, :]

        # ---- vW_h -> VB rows h (independent of attention; DMAs start early)
        for h in range(8):
            vw_ps = pp.tile([128, 512], f32, tag="ps")
            nc.tensor.matmul(vw_ps[:], hslice(vhT, h, slice(0, 128)),
                             hslice(fwT_s, h, None), start=True, stop=True)
            vw_sb = stage.tile([128, 512], f32, tag="vw_sb", bufs=6)
            nc.scalar.copy(vw_sb[:], vw_ps[:])
            for g in range(4):
                nc.sync.dma_start(VB[32 * g + h: 32 * g + h + 1, :, :],
                                  vw_sb[32 * g: 32 * g + 32, :])

        # ---- qattn scores + softmax (transposes emitted later)
        qaexps = []
        for h in range(8):
            ps_q = pp.tile([64, 256], f32, tag="ps")
            nc.tensor.matmul(ps_q[:], hslice(quhT, h, None),
                             hslice(khT, h, None), start=True, stop=True)
            qnmax8 = small.tile([64, 1], f32, tag="qnmax8")
            nc.vector.tensor_reduce(qnmax8[:], ps_q[:], mybir.AxisListType.X,
                                    Alu.max, negate=True)
            nc.vector.tensor_scalar_mul(qnmax8[:], qnmax8[:], 0.125)
            qaexp = stage.tile([64, 256], f32, tag="qaexp", bufs=8)
            qrsum = small.tile([64, 1], f32, tag="qrsum")
            nc.scalar.activation(qaexp[:], ps_q[:], Act.Exp,
                                 bias=qnmax8[:], scale=0.125,
                                 accum_out=qrsum[:])
            qrrec = small.tile([64, 1], f32, tag="qrrec")
            nc.vector.reciprocal(qrrec[:], qrsum[:])
            nc.vector.tensor_scalar_mul(qaexp[:], qaexp[:], qrrec[:])
            qaexps.append(qaexp)

        # ---- attention scores + softmax
        aexps, rrecs = [], []
        for h in range(8):
            ps_s = pp.tile([128, 256], f32, tag="ps")
            nc.tensor.matmul(ps_s[:], hslice(qhT, h, None),
                             hslice(khT, h, None), start=True, stop=True)
            nmax8 = small.tile([128, 1], f32, tag="nmax8")
            nc.vector.tensor_reduce(nmax8[:], ps_s[:], mybir.AxisListType.X,
                                    Alu.max, negate=True)
            nc.vector.tensor_scalar_mul(nmax8[:], nmax8[:], 0.125)
            aexp = stage.tile([128, 256], f32r, tag="aexp", bufs=8)
            rsum = small.tile([128, 1], f32, tag="rsum")
            nc.scalar.activation(aexp[:], ps_s[:], Act.Exp,
                                 bias=nmax8[:], scale=0.125, accum_out=rsum[:])
            rrec = small.tile([128, 1], f32, tag="rrec")
            nc.vector.reciprocal(rrec[:], rsum[:])
            aexps.append(aexp)
            rrecs.append(rrec)

        # ---- attention scores + softmax
        aexps, rrecs = [], []
        for h in range(8):
            ps_s = pp.tile([128, 256], f32, tag="ps")
            nc.tensor.matmul(ps_s[:], hslice(qhT, h, None),
                             hslice(khT, h, None), start=True, stop=True)
            nmax8 = small.tile([128, 1], f32, tag="nmax8")
            nc.vector.tensor_reduce(nmax8[:], ps_s[:], mybir.AxisListType.X,
                                    Alu.max, negate=True)
            nc.vector.tensor_scalar_mul(nmax8[:], nmax8[:], 0.125)
            aexp = stage.tile([128, 256], f32r, tag="aexp", bufs=8)
            rsum = small.tile([128, 1], f32, tag="rsum")
            nc.scalar.activation(aexp[:], ps_s[:], Act.Exp,
                                 bias=nmax8[:], scale=0.125, accum_out=rsum[:])
            rrec = small.tile([128, 1], f32, tag="rrec")
            nc.vector.reciprocal(rrec[:], rsum[:])
            aexps.append(aexp)
            rrecs.append(rrec)

        # ---- qattn -> QBT on-chip: interleave columns, then PE transpose.
        # qcat[:, sh, 32g + h] = qattn[h, nq, 32g + 16*half + sh] (f32r)
        for half in range(2):
            qcat = stage.tile([64, 16, 128], f32r, name=f"qcat{half}",
                              tag="qcat", bufs=1)
            nc.vector.memset(qcat[:].bitcast(f32), 1.0)
            for h in range(8):
                for g in range(4):
                    nc.vector.tensor_copy(
                        qcat[:, :, 32 * g + h],
                        qaexps[h][:, 32 * g + 16 * half:
                                   32 * g + 16 * half + 16])
            for sh in range(16):
                s = 16 * half + sh
                qt_ps = pp.tile([128, 64], f32r, tag="ps")
                nc.tensor.transpose(qt_ps[:], qcat[:, sh, :], id_s[:64, :64])
                nc.vector.tensor_copy(QBT[:, s, :], qt_ps[:])

        aTs = []
        for h in range(8):
            pair = []
            for lj in range(2):
                aT_ps = pp.tile([128, 128], f32r, tag="ps")
                nc.tensor.transpose(
                    aT_ps[:], aexps[h][:, lj * 128:(lj + 1) * 128], id_s[:])
                aT_sb = stage.tile([128, 128], f32r, tag="aT", bufs=10)
                nc.vector.tensor_copy(aT_sb[:], aT_ps[:])
                pair.append(aT_sb)
            aTs.append(pair)
        outns = []
        for h in range(8):
            out_ps = pp.tile([128, 64], f32, tag="ps")
            for lj in range(2):
                nc.tensor.matmul(out_ps[:], aTs[h][lj][:],
                                 vh[lj][:, h * 64:(h + 1) * 64],
                                 start=(lj == 0), stop=(lj == 1))
            outn = stage.tile([128, 64], f32r, tag="outn", bufs=8)
            nc.vector.tensor_scalar_mul(outn[:], out_ps[:], rrecs[h][:])
            outns.append(outn)
        for h in range(8):
            oT_ps = pp.tile([64, 128], f32r, tag="ps")
            nc.tensor.transpose(oT_ps[:], outns[h][:], id_s[:])
            nc.scalar.copy(outT[(h % 2) * 64:(h % 2) * 64 + 64, h // 2, :],
                           oT_ps[:])

        # ---- outW + base
        ow_ps = pp.tile([128, 512], f32, tag="ps")
        for c in range(4):
            nc.tensor.matmul(ow_ps[:], outT[:, c, :], fwT_s[:, c, :],
                             start=(c == 0), stop=(c == 3))
        base = const.tile([128, 512], f32, tag="base")
        nc.vector.tensor_add(base[:], ow_ps[:], qpb_s[:])
        for g in range(4):
            nc.sync.dma_start(VB[32 * g + 8: 32 * g + 9, :, :],
                              base[32 * g: 32 * g + 32, :])

        # ---- termB + layernorm: 16 passes, 8 lk each (4 row x 2 col tiles)
        pp_cm.__exit__(None, None, None)
        with tc.tile_pool(name="pb", bufs=8, space="PSUM") as pb:
            for u in range(16):
                banks = []
                for g in range(4):
                    bank = pb.tile([128, 512], f32, name=f"bank{u}_{g}",
                                   tag="bank")
                    banks.append(bank)
                    for c in range(2):
                        s = 2 * u + c
                        nc.tensor.matmul(
                            bank[64 * c: 64 * c + 64, :],
                            QBT[32 * g: 32 * g + 9, s, :],
                            VB[32 * g: 32 * g + 9, s, :],
                            start=True, stop=True,
                            tile_position=(32 * g, 64 * c))
                st6 = small.tile([128, 6, 4], f32, tag="st6")
                st2 = small.tile([128, 2, 4], f32, tag="st2")
                for g in range(4):
                    nc.vector.bn_stats(st6[:, :, g:g + 1], banks[g][:])
                    nc.vector.bn_aggr(st2[:, :, g:g + 1], st6[:, :, g:g + 1])
                std4 = small.tile([128, 4], f32, tag="std4")
                nc.scalar.activation(std4[:], st2[:, 1, :], Act.Sqrt,
                                     bias=eps_t[:], scale=1.0)
                rstd4 = small.tile([128, 4], f32, tag="rstd4")
                nc.vector.reciprocal(rstd4[:], std4[:])
                nmr4 = small.tile([128, 4], f32, tag="nmr4")
                nc.vector.scalar_tensor_tensor(nmr4[:], st2[:, 0, :], -1.0,
                                               rstd4[:], op0=Alu.mult,
                                               op1=Alu.mult)
                big = stage.tile([128, 2048], f32, tag="big", bufs=2)
                for g in range(4):
                    nc.scalar.activation(big[:, 512 * g: 512 * g + 512],
                                         banks[g][:], Act.Identity,
                                         bias=nmr4[:, g:g + 1],
                                         scale=rstd4[:, g:g + 1])
                    if apply_affine:
                        nc.vector.tensor_mul(big[:, 512 * g: 512 * g + 512],
                                             big[:, 512 * g: 512 * g + 512],
                                             lnw_s[:])
                        nc.vector.tensor_add(big[:, 512 * g: 512 * g + 512],
                                             big[:, 512 * g: 512 * g + 512],
                                             lnb_s[:])
                nc.sync.dma_start(outd[u], big[:])

    nc.compile()
    return nc


def _get_nc(apply_affine: bool):
    if apply_affine not in _CACHE:
        _CACHE[apply_affine] = _build(apply_affine)
    return _CACHE[apply_affine]


def _decode_out(raw):
    # [u, c, nq, g, m] -> [nq, lk = 32g + 2u + c, m]
    arr = np.asarray(raw).reshape(16, 2, 64, 4, 512)
    return arr.transpose(2, 3, 0, 1, 4).reshape(NQ, 128, 512)


def _to_dev(x):
    # [512, N] -> [128, 4, N]
    return np.ascontiguousarray(
        x.reshape(4, 128, x.shape[1]).transpose(1, 0, 2))


def make_in_maps(q, k, v, query, w_qs, w_ks, w_vs, w_query, fc_w, fc_b,
                 ln_w, ln_b, apply_affine):
    ident = np.eye(128, dtype=np.float32)
    wqT = _to_dev(w_qs.T)
    wkT = _to_dev(w_ks.T)
    wvT = _to_dev(w_vs.T)
    wuT = _to_dev(w_query.T)
    fwT = _to_dev(fc_w.T)
    in_maps = []
    for c in range(N_CORES):
        b, half = c // 2, c % 2
        L0 = 128 * half
        # swap lk halves so that local lk 0..127 is always this core's half
        perm = np.r_[L0:L0 + 128, (128 - L0):(128 - L0) + 128]
        m = {
            "kT": _to_dev(np.ascontiguousarray(k[b][perm].T)),
            "vT": _to_dev(np.ascontiguousarray(v[b][perm].T)),
            "qT": _to_dev(np.ascontiguousarray(q[b, L0:L0 + 128].T)),
            "quT": _to_dev(query[b].T),
            "wq": wqT, "wk": wkT, "wv": wvT, "wu": wuT, "fwT": fwT,
            "qpb": q[b, L0:L0 + 128] + fc_b,
            "ident": ident,
        }
        if apply_affine:
            m["lnw_r"] = np.ascontiguousarray(
                np.broadcast_to(ln_w, (128, 512)))
            m["lnb_r"] = np.ascontiguousarray(
                np.broadcast_to(ln_b, (128, 512)))
        in_maps.append(m)
    return in_maps


def kernel(q, k, v, query, w_qs, w_ks, w_vs, w_query, fc_w, fc_b, ln_w, ln_b):
    from concourse.bass_utils import run_bass_kernel_spmd

    q, k, v, query = (np.asarray(a, np.float32) for a in (q, k, v, query))
    w_qs, w_ks, w_vs, w_query, fc_w = (
        np.asarray(a, np.float32) for a in (w_qs, w_ks, w_vs, w_query, fc_w))
    fc_b, ln_w, ln_b = (np.asarray(a, np.float32)
                        for a in (fc_b, ln_w, ln_b))

    apply_affine = not (np.all(ln_w == 1.0) and np.all(ln_b == 0.0))
    nc = _get_nc(apply_affine)
    in_maps = make_in_maps(q, k, v, query, w_qs, w_ks, w_vs, w_query,
                           fc_w, fc_b, ln_w, ln_b, apply_affine)
    res = run_bass_kernel_spmd(nc, in_maps, list(range(N_CORES)))

    full = np.empty((B, NQ, LEN_K, D_MODEL), np.float32)
    for c in range(N_CORES):
        b, half = c // 2, c % 2
        full[b, :, 128 * half:128 * half + 128, :] = _decode_out(
            res.results[c]["out"])
    return full


# revision 35
# speedup vs baseline: 1.5187x; 1.0835x over previous
"""Trainium2 Bass kernel for nn_MultiHeadQuery_selfattention.

Sharding: 8 cores = 4 batches x 2 lk-halves. Core c handles batch b=c//2 and
key rows L = [128*(c%2), 128*(c%2)+128). Each core computes its output slice
out[b, :, L, :] fully independently (no collectives). A single NEFF serves
all cores: for the upper half, the host swaps the two lk-halves of k and v
(softmax/attention sums are permutation-invariant over keys), so on device
"local lk 0..127" is always the core's half.

Math per core (restructured from the reference):
  khT/vhT/qhT/quT: per-head projections kept transposed [head_dim, token]
  attn_h  = softmax(qh_h @ kh_h.T / 8)          rows = my 128 lk
  out_h   = attn_h @ vh_h                       [128, 64]
  outW    = concat_h(out_h) @ fc_w.T            [128, 512]
  base    = outW + q[b, L, :] + fc_b            [128, 512]
  vW_h    = vh_h[L] @ fc_w.T[64h:64h+64]        [128, 512]
  qattn_h = softmax(qu_h @ kh_h.T / 8)          [64, 256]
  res[nq, l, m] = base[l, m] + sum_h qattn[h, nq, l] * vW[h, l, m]
  out = layernorm(res) * ln_w + ln_b

res is produced on the PE as 128 small matmuls with K=9: stationary =
[qattn rows; ones], moving = [vW rows; base], so PSUM holds res directly.
lk pairs share one PSUM bank [128, 512] = (2 lk x 64 nq) rows; LN stats via
bn_stats/bn_aggr; normalize on ACT. QB/VB hold the K=9 operands in 4
partition groups (local lk l -> partitions 32*(l//32)+{0..8}, slot l%32).
"""

import numpy as np

N_HEAD, D_MODEL, D_K, D_V = 8, 512, 64, 64
B, LEN_K, NQ = 4, 256, 64
LN_EPS = 1e-5
N_CORES = 8

_CACHE = {}


def _build(apply_affine: bool):
    import contextlib

    import concourse.tile as tile
    from concourse import bacc, mybir

    f32 = mybir.dt.float32
    f32r = mybir.dt.float32r
    Alu = mybir.AluOpType
    Act = mybir.ActivationFunctionType

    nc = bacc.Bacc("TRN2", target_bir_lowering=False, debug=False,
                   num_devices=N_CORES)

    def din(name, shape, dt=None):
        return nc.dram_tensor(name, shape, dt or f32, kind="ExternalInput")

    # device-layout inputs ([512, N] host tensors stored as [128, 4, N])
    kT = din("kT", [128, 4, 256], f32r)
    vT = din("vT", [128, 4, 256], f32r)
    qT = din("qT", [128, 4, 128], f32r)     # only my 128 lq columns
    quT = din("quT", [128, 4, 64], f32r)
    wq = din("wq", [128, 4, 512], f32r)     # w_qs.T
    wk = din("wk", [128, 4, 512], f32r)
    wv = din("wv", [128, 4, 512], f32r)
    wu = din("wu", [128, 4, 512], f32r)
    fwT = din("fwT", [128, 4, 512], f32r)   # fc_w.T
    qpb = din("qpb", [128, 512])      # q[b, L, :] + fc_b
    ident = din("ident", [128, 128], f32r)
    if apply_affine:
        lnw_r = din("lnw_r", [128, 512])
        lnb_r = din("lnb_r", [128, 512])
    # pass u, column-block g holds rows (lkk, nq) of lk = 32g + 2u + lkk
    outd = nc.dram_tensor("out", [16, 128, 2048], f32, kind="ExternalOutput")
    VD = nc.dram_tensor("VD", [128, 8, 512], f32)  # vW bounce buffer

    with tile.TileContext(nc) as tc, contextlib.ExitStack() as ctx:
        const = ctx.enter_context(tc.tile_pool(name="const", bufs=1))
        stage = ctx.enter_context(tc.tile_pool(name="stage", bufs=4))
        small = ctx.enter_context(tc.tile_pool(name="small", bufs=8))
        pp_cm = tc.tile_pool(name="pp", bufs=8, space="PSUM")
        pp = pp_cm.__enter__()

        def load(dram, shape):
            t = const.tile(shape, dram.dtype, tag=dram.name)
            nc.sync.dma_start(t[:], dram[:])
            return t

        kT_s = load(kT, [128, 4, 256])
        vT_s = load(vT, [128, 4, 256])
        qT_s = load(qT, [128, 4, 128])
        quT_s = load(quT, [128, 4, 64])
        wq_s = load(wq, [128, 4, 512])
        wk_s = load(wk, [128, 4, 512])
        wv_s = load(wv, [128, 4, 512])
        wu_s = load(wu, [128, 4, 512])
        fwT_s = load(fwT, [128, 4, 512])
        qpb_s = load(qpb, [128, 512])
        id_s = load(ident, [128, 128])
        if apply_affine:
            lnw_s = load(lnw_r, [128, 512])
            lnb_s = load(lnb_r, [128, 512])

        # ---- projections: xhT[hd, tok] = sum_dm w.T[dm, hd] * x.T[dm, tok]
        def project(w_s, x_s, ncols, tag):
            dst = const.tile([128, 4, ncols], f32r, tag=tag)
            for mo in range(4):
                ps = pp.tile([128, ncols], f32, tag="ps")
                for ki in range(4):
                    nc.tensor.matmul(
                        ps[:], w_s[:, ki, mo * 128:(mo + 1) * 128],
                        x_s[:, ki, :], start=(ki == 0), stop=(ki == 3))
                nc.scalar.copy(dst[:, mo, :], ps[:])
            return dst

        khT = project(wk_s, kT_s, 256, "khT")
        vhT = project(wv_s, vT_s, 256, "vhT")
        qhT = project(wq_s, qT_s, 128, "qhT")
        quhT = project(wu_s, quT_s, 64, "quhT")

        # ---- vh untransposed [lk, hd] as two [128, 512] tiles
        vh = [const.tile([128, 512], f32r, name=f"vh{j}", tag=f"vh{j}")
              for j in range(2)]
        for lj in range(2):
            for hc in range(4):
                ps = pp.tile([128, 128], f32r, tag="ps")
                nc.tensor.transpose(
                    ps[:], vhT[:, hc, lj * 128:(lj + 1) * 128], id_s[:])
                nc.scalar.copy(vh[lj][:, hc * 128:(hc + 1) * 128], ps[:])

        # ---- per-head attention
        outT = const.tile([128, 4, 128], f32r, tag="outT")  # [hd, lq]
        eps_t = const.tile([128, 1], f32, tag="eps_t")
        nc.vector.memset(eps_t[:], LN_EPS)
        VB = const.tile([128, 32, 512], f32, tag="VB")
        QBT = const.tile([128, 32, 64], f32, tag="QBT")


        def hslice(t, h, cols):
            po = (h % 2) * 64
            return t[po:po + 64, h // 2, cols] if cols is not None else \
                t[po:po + 64, h // 2, :]

        # ---- vW_h -> VB rows h (independent of attention; DMAs start early)
        for h in range(8):
            vw_ps = pp.tile([128, 512], f32, tag="ps")
            nc.tensor.matmul(vw_ps[:], hslice(vhT, h, slice(0, 128)),
                             hslice(fwT_s, h, None), start=True, stop=True)
            vw_sb = stage.tile([128, 512], f32, tag="vw_sb", bufs=6)
            nc.scalar.copy(vw_sb[:], vw_ps[:])
            nc.sync.dma_start(VD[:, h, :], vw_sb[:])
        for g in range(4):
            nc.sync.dma_start(
                VB[32 * g: 32 * g + 8, :, :],
                VD[32 * g: 32 * g + 32, :, :].rearrange("s h m -> h s m"))

        # ---- qattn scores + softmax (transposes emitted later)
        qaexps = []
        for h in range(8):
            ps_q = pp.tile([64, 256], f32, tag="ps")
            nc.tensor.matmul(ps_q[:], hslice(quhT, h, None),
                             hslice(khT, h, None), start=True, stop=True)
            qnmax8 = small.tile([64, 1], f32, tag="qnmax8")
            nc.vector.tensor_reduce(qnmax8[:], ps_q[:], mybir.AxisListType.X,
                                    Alu.max, negate=True)
            nc.vector.tensor_scalar_mul(qnmax8[:], qnmax8[:], 0.125)
            qaexp = stage.tile([64, 256], f32, tag="qaexp", bufs=8)
            qrsum = small.tile([64, 1], f32, tag="qrsum")
            nc.scalar.activation(qaexp[:], ps_q[:], Act.Exp,
                                 bias=qnmax8[:], scale=0.125,
                                 accum_out=qrsum[:])
            qrrec = small.tile([64, 1], f32, tag="qrrec")
            nc.vector.reciprocal(qrrec[:], qrsum[:])
            nc.vector.tensor_scalar_mul(qaexp[:], qaexp[:], qrrec[:])
            qaexps.append(qaexp)

        # ---- attention scores + softmax
        aexps, rrecs = [], []
        for h in range(8):
            ps_s = pp.tile([128, 256], f32, tag="ps")
            nc.tensor.matmul(ps_s[:], hslice(qhT, h, None),
                             hslice(khT, h, None), start=True, stop=True)
            nmax8 = small.tile([128, 1], f32, tag="nmax8")
            nc.vector.tensor_reduce(nmax8[:], ps_s[:], mybir.AxisListType.X,
                                    Alu.max, negate=True)
            nc.vector.tensor_scalar_mul(nmax8[:], nmax8[:], 0.125)
            aexp = stage.tile([128, 256], f32r, tag="aexp", bufs=8)
            rsum = small.tile([128, 1], f32, tag="rsum")
            nc.scalar.activation(aexp[:], ps_s[:], Act.Exp,
                                 bias=nmax8[:], scale=0.125, accum_out=rsum[:])
            rrec = small.tile([128, 1], f32, tag="rrec")
            nc.vector.reciprocal(rrec[:], rsum[:])
            aexps.append(aexp)
            rrecs.append(rrec)

        # ---- attention scores + softmax
        aexps, rrecs = [], []
        for h in range(8):
            ps_s = pp.tile([128, 256], f32, tag="ps")
            nc.tensor.matmul(ps_s[:], hslice(qhT, h, None),
                             hslice(khT, h, None), start=True, stop=True)
            nmax8 = small.tile([128, 1], f32, tag="nmax8")
            nc.vector.tensor_reduce(nmax8[:], ps_s[:], mybir.AxisListType.X,
                                    Alu.max, negate=True)
            nc.vector.tensor_scalar_mul(nmax8[:], nmax8[:], 0.125)
            aexp = stage.tile([128, 256], f32r, tag="aexp", bufs=8)
            rsum = small.tile([128, 1], f32, tag="rsum")
            nc.scalar.activation(aexp[:], ps_s[:], Act.Exp,
                                 bias=nmax8[:], scale=0.125, accum_out=rsum[:])
            rrec = small.tile([128, 1], f32, tag="rrec")
            nc.vector.reciprocal(rrec[:], rsum[:])
            aexps.append(aexp)
            rrecs.append(rrec)

        # ---- qattn -> QBT on-chip: interleave columns, then PE transpose.
        # qcat[:, sh, 32g + h] = qattn[h, nq, 32g + 16*half + sh] (f32r)
        for half in range(2):
            qcat = stage.tile([64, 16, 128], f32r, name=f"qcat{half}",
                              tag="qcat", bufs=1)
            nc.vector.memset(qcat[:].bitcast(f32), 1.0)
            for h in range(8):
                for g in range(4):
                    nc.vector.tensor_copy(
                        qcat[:, :, 32 * g + h],
                        qaexps[h][:, 32 * g + 16 * half:
                                   32 * g + 16 * half + 16])
            for sh in range(16):
                s = 16 * half + sh
                qt_ps = pp.tile([128, 64], f32r, tag="ps")
                nc.tensor.transpose(qt_ps[:], qcat[:, sh, :], id_s[:64, :64])
                nc.vector.tensor_copy(QBT[:, s, :], qt_ps[:])

        aTs = []
        for h in range(8):
            pair = []
            for lj in range(2):
                aT_ps = pp.tile([128, 128], f32r, tag="ps")
                nc.tensor.transpose(
                    aT_ps[:], aexps[h][:, lj * 128:(lj + 1) * 128], id_s[:])
                aT_sb = stage.tile([128, 128], f32r, tag="aT", bufs=10)
                nc.vector.tensor_copy(aT_sb[:], aT_ps[:])
                pair.append(aT_sb)
            aTs.append(pair)
        outns = []
        for h in range(8):
            out_ps = pp.tile([128, 64], f32, tag="ps")
            for lj in range(2):
                nc.tensor.matmul(out_ps[:], aTs[h][lj][:],
                                 vh[lj][:, h * 64:(h + 1) * 64],
                                 start=(lj == 0), stop=(lj == 1))
            outn = stage.tile([128, 64], f32r, tag="outn", bufs=8)
            nc.vector.tensor_scalar_mul(outn[:], out_ps[:], rrecs[h][:])
            outns.append(outn)
        for h in range(8):
            oT_ps = pp.tile([64, 128], f32r, tag="ps")
            nc.tensor.transpose(oT_ps[:], outns[h][:], id_s[:])
            nc.scalar.copy(outT[(h % 2) * 64:(h % 2) * 64 + 64, h // 2, :],
                           oT_ps[:])

        # ---- outW + base
        ow_ps = pp.tile([128, 512], f32, tag="ps")
        for c in range(4):
            nc.tensor.matmul(ow_ps[:], outT[:, c, :], fwT_s[:, c, :],
                             start=(c == 0), stop=(c == 3))
        base = const.tile([128, 512], f32, tag="base")
        nc.vector.tensor_add(base[:], ow_ps[:], qpb_s[:])
        for g in range(4):
            nc.scalar.dma_start(VB[32 * g + 8: 32 * g + 9, :, :],
                                base[32 * g: 32 * g + 32, :])

        # ---- termB + layernorm: 16 passes, 8 lk each (4 row x 2 col tiles)
        pp_cm.__exit__(None, None, None)
        with tc.tile_pool(name="pb", bufs=8, space="PSUM") as pb:
            for u in range(16):
                banks = []
                for g in range(4):
                    bank = pb.tile([128, 512], f32, name=f"bank{u}_{g}",
                                   tag="bank")
                    banks.append(bank)
                    for c in range(2):
                        s = 2 * u + c
                        nc.tensor.matmul(
                            bank[64 * c: 64 * c + 64, :],
                            QBT[32 * g: 32 * g + 9, s, :],
                            VB[32 * g: 32 * g + 9, s, :],
                            start=True, stop=True,
                            tile_position=(32 * g, 64 * c))
                st6 = small.tile([128, 6, 4], f32, tag="st6")
                st2 = small.tile([128, 2, 4], f32, tag="st2")
                for g in range(4):
                    nc.vector.bn_stats(st6[:, :, g:g + 1], banks[g][:])
                    nc.vector.bn_aggr(st2[:, :, g:g + 1], st6[:, :, g:g + 1])
                std4 = small.tile([128, 4], f32, tag="std4")
                nc.scalar.activation(std4[:], st2[:, 1, :], Act.Sqrt,
                                     bias=eps_t[:], scale=1.0)
                rstd4 = small.tile([128, 4], f32, tag="rstd4")
                nc.vector.reciprocal(rstd4[:], std4[:])
                nmr4 = small.tile([128, 4], f32, tag="nmr4")
                nc.vector.scalar_tensor_tensor(nmr4[:], st2[:, 0, :], -1.0,
                                               rstd4[:], op0=Alu.mult,
                                               op1=Alu.mult)
                big = stage.tile([128, 2048], f32, tag="big", bufs=2)
                for g in range(4):
                    nc.scalar.activation(big[:, 512 * g: 512 * g + 512],
                                         banks[g][:], Act.Identity,
                                         bias=nmr4[:, g:g + 1],
                                         scale=rstd4[:, g:g + 1])
                    if apply_affine:
                        nc.vector.tensor_mul(big[:, 512 * g: 512 * g + 512],
                                             big[:, 512 * g: 512 * g + 512],
                                             lnw_s[:])
                        nc.vector.tensor_add(big[:, 512 * g: 512 * g + 512],
                                             big[:, 512 * g: 512 * g + 512],
                                             lnb_s[:])
                nc.sync.dma_start(outd[u], big[:])

    nc.compile()
    return nc


def _get_nc(apply_affine: bool):
    if apply_affine not in _CACHE:
        _CACHE[apply_affine] = _build(apply_affine)
    return _CACHE[apply_affine]


def _decode_out(raw):
    # [u, c, nq, g, m] -> [nq, lk = 32g + 2u + c, m]
    arr = np.asarray(raw).reshape(16, 2, 64, 4, 512)
    return arr.transpose(2, 3, 0, 1, 4).reshape(NQ, 128, 512)


def _to_dev(x):
    # [512, N] -> [128, 4, N]
    return np.ascontiguousarray(
        x.reshape(4, 128, x.shape[1]).transpose(1, 0, 2))


def make_in_maps(q, k, v, query, w_qs, w_ks, w_vs, w_query, fc_w, fc_b,
                 ln_w, ln_b, apply_affine):
    ident = np.eye(128, dtype=np.float32)
    wqT = _to_dev(w_qs.T)
    wkT = _to_dev(w_ks.T)
    wvT = _to_dev(w_vs.T)
    wuT = _to_dev(w_query.T)
    fwT = _to_dev(fc_w.T)
    in_maps = []
    for c in range(N_CORES):
        b, half = c // 2, c % 2
        L0 = 128 * half
        # swap lk halves so that local lk 0..127 is always this core's half
        perm = np.r_[L0:L0 + 128, (128 - L0):(128 - L0) + 128]
        m = {
            "kT": _to_dev(np.ascontiguousarray(k[b][perm].T)),
            "vT": _to_dev(np.ascontiguousarray(v[b][perm].T)),
            "qT": _to_dev(np.ascontiguousarray(q[b, L0:L0 + 128].T)),
            "quT": _to_dev(query[b].T),
            "wq": wqT, "wk": wkT, "wv": wvT, "wu": wuT, "fwT": fwT,
            "qpb": q[b, L0:L0 + 128] + fc_b,
            "ident": ident,
        }
        if apply_affine:
            m["lnw_r"] = np.ascontiguousarray(
                np.broadcast_to(ln_w, (128, 512)))
            m["lnb_r"] = np.ascontiguousarray(
                np.broadcast_to(ln_b, (128, 512)))
        in_maps.append(m)
    return in_maps


def kernel(q, k, v, query, w_qs, w_ks, w_vs, w_query, fc_w, fc_b, ln_w, ln_b):
    from concourse.bass_utils import run_bass_kernel_spmd

    q, k, v, query = (np.asarray(a, np.float32) for a in (q, k, v, query))
    w_qs, w_ks, w_vs, w_query, fc_w = (
        np.asarray(a, np.float32) for a in (w_qs, w_ks, w_vs, w_query, fc_w))
    fc_b, ln_w, ln_b = (np.asarray(a, np.float32)
                        for a in (fc_b, ln_w, ln_b))

    apply_affine = not (np.all(ln_w == 1.0) and np.all(ln_b == 0.0))
    nc = _get_nc(apply_affine)
    in_maps = make_in_maps(q, k, v, query, w_qs, w_ks, w_vs, w_query,
                           fc_w, fc_b, ln_w, ln_b, apply_affine)
    res = run_bass_kernel_spmd(nc, in_maps, list(range(N_CORES)))

    full = np.empty((B, NQ, LEN_K, D_MODEL), np.float32)
    for c in range(N_CORES):
        b, half = c // 2, c % 2
        full[b, :, 128 * half:128 * half + 128, :] = _decode_out(
            res.results[c]["out"])
    return full
